# revision 1
# baseline (speedup 1.0000x reference)
"""Trainium2 Bass kernel for the HNN sparse-MLP network.

Strategy: the sparse layers have fixed connectivity, so we densify the
sparse weight lists into dense matrices on the host and run the whole
network as dense fp32r matmuls on the tensor engine, data-parallel over
the batch across 8 NeuronCores (1024 rows each).

Layout: activations live feature-on-partition ([features, batch]) the
whole way through, so no transposes are needed between layers:
    h_out[f_out, b] = relu( sum_k W[f_in, f_out]^T . h_in[f_in, b] + bias )
with lhsT = W k-tile [128, Mw], rhs = h_in k-tile [128, 512].

The scalar fc taps (fc1..fc4) are folded in as one extra output feature
per layer (an Mw=1 matmul tile); the final readout is a K=4 matmul over
the concatenated taps.

fp32r (fp32 rounded to 11-bit mantissa) runs the PE at full rate
(1 col/cycle, 4x faster than plain fp32) at ~1e-4 relative error.
Inputs are pre-rounded on the host so DMAs can feed fp32r tiles
directly.
"""

import sys

sys.path.insert(0, "/opt/trn_rl_repo")

import numpy as np

import concourse.bass as bass
import concourse.tile as tile
import concourse.mybir as mybir
from concourse import bacc, bass_utils

F32 = mybir.dt.float32
F32R = mybir.dt.float32r
RELU = mybir.ActivationFunctionType.Relu
COPY = mybir.ActivationFunctionType.Copy

NCORES = 8
B, L1, L2, L3, L4 = 8192, 4096, 2048, 1024, 512
BC = B // NCORES          # batch rows per core
NB = 512                  # matmul moving free dim (PSUM bank limit for fp32)
NBLK = BC // NB           # N-blocks per core


def round_fp32r(a: np.ndarray) -> np.ndarray:
    """Round fp32 to fp32r (11-bit mantissa, RNE) = walrus fp32_to_fp32r."""
    u = np.ascontiguousarray(a, dtype=np.float32).view(np.uint32)
    lsb = (u >> 12) & 1
    r = (u + 0x7FF + lsb) & np.uint32(0xFFFFF000)
    return r.view(np.float32)


def _densify(w, out_idx, in_idx, fc_w, in_dim, out_dim):
    """Dense [in_dim, out_dim+1] matrix from edge lists + fc column."""
    wd = np.zeros((in_dim, out_dim + 1), np.float32)
    np.add.at(wd, (np.asarray(in_idx), np.asarray(out_idx)), np.asarray(w, np.float32))
    wd[:, out_dim] = np.asarray(fc_w, np.float32).reshape(-1)
    return wd


def _pack_w(wd, in_dim, out_dim):
    """Pack dense [in_dim, out_dim+1] into per-M-block contiguous tiles.

    Returns (wp [T, 128, K/128*128], wfc [128, K/128], T) where
    wp[t, p, j*128+m] = wd[j*128+p, t*128+m] and wfc[p, j] = wd[j*128+p, out_dim].
    """
    kt = in_dim // 128
    t = out_dim // 128
    wmain = wd[:, :out_dim].reshape(kt, 128, t, 128)
    wp = np.ascontiguousarray(wmain.transpose(2, 1, 0, 3).reshape(t, 128, kt * 128))
    wfc = np.ascontiguousarray(wd[:, out_dim].reshape(kt, 128).T)
    return round_fp32r(wp), round_fp32r(wfc), t


def _pack_b(b, fc_b, out_dim):
    """Pack bias [out_dim] + fc bias into [128, T+1] (column t = tile t)."""
    t = out_dim // 128
    bp = np.zeros((128, t + 1), np.float32)
    bp[:, :t] = np.asarray(b, np.float32).reshape(t, 128).T
    bp[0, t] = float(np.asarray(fc_b).reshape(-1)[0])
    return bp


def _build_program():
    nc = bacc.Bacc("TRN2", target_bir_lowering=False, debug=False,
                   num_devices=NCORES)
    d = {}
    d["xt"] = nc.dram_tensor("xt", [L1, BC], F32R, kind="ExternalInput").ap()
    d["w1p"] = nc.dram_tensor("w1p", [16, 128, L1], F32R, kind="ExternalInput").ap()
    d["w1fc"] = nc.dram_tensor("w1fc", [128, 32], F32R, kind="ExternalInput").ap()
    d["b1"] = nc.dram_tensor("b1", [128, 17], F32, kind="ExternalInput").ap()
    d["w2p"] = nc.dram_tensor("w2p", [8, 128, L2], F32R, kind="ExternalInput").ap()
    d["w2fc"] = nc.dram_tensor("w2fc", [128, 16], F32R, kind="ExternalInput").ap()
    d["b2"] = nc.dram_tensor("b2", [128, 9], F32, kind="ExternalInput").ap()
    d["w3p"] = nc.dram_tensor("w3p", [4, 128, L3], F32R, kind="ExternalInput").ap()
    d["w3fc"] = nc.dram_tensor("w3fc", [128, 8], F32R, kind="ExternalInput").ap()
    d["b3"] = nc.dram_tensor("b3", [128, 5], F32, kind="ExternalInput").ap()
    d["w4"] = nc.dram_tensor("w4", [128, 4], F32R, kind="ExternalInput").ap()
    d["fc4b"] = nc.dram_tensor("fc4b", [1, 1], F32, kind="ExternalInput").ap()
    d["rw"] = nc.dram_tensor("rw", [4, 1], F32R, kind="ExternalInput").ap()
    d["rb"] = nc.dram_tensor("rb", [1, 1], F32, kind="ExternalInput").ap()
    out_d = nc.dram_tensor("out", [1, BC], F32, kind="ExternalOutput").ap()

    with tile.TileContext(nc) as tc:
        _emit(nc, tc, d, out_d)
    nc.compile()
    return nc


def _emit(nc, tc, d, out_d):
    from contextlib import ExitStack

    with ExitStack() as ctx:
        consts = ctx.enter_context(tc.tile_pool(name="consts", bufs=1))
        psum = ctx.enter_context(tc.tile_pool(name="psum", bufs=4, space="PSUM"))
        stage = ctx.enter_context(tc.tile_pool(name="stage", bufs=4))
        dram = ctx.enter_context(tc.tile_pool(name="dram", bufs=1, space="DRAM"))

        def cload(name, shape, dt):
            t = consts.tile(shape, dt, tag=name)
            nc.sync.dma_start(t[:], d[name][:])
            return t

        b1sb = cload("b1", [128, 17], F32)
        b2sb = cload("b2", [128, 9], F32)
        b3sb = cload("b3", [128, 5], F32)
        w1fc = cload("w1fc", [128, 32], F32R)
        w2fc = cload("w2fc", [128, 16], F32R)
        w3fc = cload("w3fc", [128, 8], F32R)
        w4sb = cload("w4", [128, 4], F32R)
        fc4b = cload("fc4b", [1, 1], F32)
        rwsb = cload("rw", [4, 1], F32R)
        rbsb = cload("rb", [1, 1], F32)

        h1d = dram.tile([17 * 128, BC], F32R)

        # ---- layer 1: x [4096, BC] -> h1 [2049, BC] (spilled to DRAM) ----
        with tc.tile_pool(name="xts", bufs=32) as xpool, \
             tc.tile_pool(name="w1m", bufs=2) as w1pool:
            xts = []
            xview = d["xt"].rearrange("(j p) b -> p j b", p=128)
            for j in range(32):
                xt = xpool.tile([128, BC], F32R, tag="xts")
                nc.sync.dma_start(xt[:], xview[:, j, :])
                xts.append(xt)

            for m in range(17):
                if m < 16:
                    mw = 128
                    wm = w1pool.tile([128, 32 * 128], F32R, tag="w1m")
                    nc.sync.dma_start(wm[:], d["w1p"][m])
                else:
                    mw = 1
                    wm = w1fc
                for nb in range(NBLK):
                    pt = psum.tile([128, NB], F32)
                    for k in range(32):
                        nc.tensor.matmul(
                            pt[:mw], wm[:, k * mw:(k + 1) * mw],
                            xts[k][:, nb * NB:(nb + 1) * NB],
                            start=(k == 0), stop=(k == 31))
                    st = stage.tile([128, NB], F32R, tag="stage")
                    nc.scalar.activation(st[:mw], pt[:mw], RELU,
                                         bias=b1sb[:mw, m:m + 1])
                    nc.sync.dma_start(
                        h1d[m * 128:m * 128 + mw, nb * NB:(nb + 1) * NB],
                        st[:mw])

        # ---- layer 2: h1 [2048, BC] -> h2 [1025, BC] (SBUF-resident) ----
        h2pool = ctx.enter_context(tc.tile_pool(name="h2", bufs=9))
        h2ts = [h2pool.tile([128, BC], F32R, tag="h2", name=f"h2_{i}") for i in range(9)]
        with tc.tile_pool(name="h1ts", bufs=16) as h1pool, \
             tc.tile_pool(name="w2m", bufs=2) as w2pool:
            h1ts = []
            for j in range(16):
                t = h1pool.tile([128, BC], F32R, tag="h1ts")
                nc.sync.dma_start(t[:], h1d[j * 128:(j + 1) * 128, :])
                h1ts.append(t)

            for m in range(9):
                if m < 8:
                    mw = 128
                    wm = w2pool.tile([128, 16 * 128], F32R, tag="w2m")
                    nc.sync.dma_start(wm[:], d["w2p"][m])
                else:
                    mw = 1
                    wm = w2fc
                for nb in range(NBLK):
                    pt = psum.tile([128, NB], F32)
                    for k in range(16):
                        nc.tensor.matmul(
                            pt[:mw], wm[:, k * mw:(k + 1) * mw],
                            h1ts[k][:, nb * NB:(nb + 1) * NB],
                            start=(k == 0), stop=(k == 15))
                    nc.scalar.activation(
                        h2ts[m][:mw, nb * NB:(nb + 1) * NB], pt[:mw], RELU,
                        bias=b2sb[:mw, m:m + 1])

        # ---- layer 3: h2 [1024, BC] -> h3 [513, BC] ----
        h3pool = ctx.enter_context(tc.tile_pool(name="h3", bufs=5))
        h3ts = [h3pool.tile([128, BC], F32R, tag="h3", name=f"h3_{i}") for i in range(5)]
        with tc.tile_pool(name="w3m", bufs=2) as w3pool:
            for m in range(5):
                if m < 4:
                    mw = 128
                    wm = w3pool.tile([128, 8 * 128], F32R, tag="w3m")
                    nc.sync.dma_start(wm[:], d["w3p"][m])
                else:
                    mw = 1
                    wm = w3fc
                for nb in range(NBLK):
                    pt = psum.tile([128, NB], F32)
                    for k in range(8):
                        nc.tensor.matmul(
                            pt[:mw], wm[:, k * mw:(k + 1) * mw],
                            h2ts[k][:, nb * NB:(nb + 1) * NB],
                            start=(k == 0), stop=(k == 7))
                    nc.scalar.activation(
                        h3ts[m][:mw, nb * NB:(nb + 1) * NB], pt[:mw], RELU,
                        bias=b3sb[:mw, m:m + 1])

        # ---- fc4 tap: h3 [512, BC] -> f4 [1, BC] ----
        f4sb = consts.tile([1, BC], F32R, tag="f4")
        for nb in range(NBLK):
            pt = psum.tile([128, NB], F32)
            for k in range(4):
                nc.tensor.matmul(pt[:1], w4sb[:, k:k + 1],
                                 h3ts[k][:, nb * NB:(nb + 1) * NB],
                                 start=(k == 0), stop=(k == 3))
            nc.scalar.activation(f4sb[:1, nb * NB:(nb + 1) * NB], pt[:1],
                                 RELU, bias=fc4b[:1])

        # ---- readout: out = ro_w . [f1 f2 f3 f4] + ro_b ----
        cat = consts.tile([4, BC], F32R, tag="cat")
        nc.sync.dma_start(cat[0:1, :], h1d[16 * 128:16 * 128 + 1, :])
        nc.sync.dma_start(cat[1:2, :], h2ts[8][0:1, :])
        nc.sync.dma_start(cat[2:3, :], h3ts[4][0:1, :])
        nc.sync.dma_start(cat[3:4, :], f4sb[0:1, :])
        outsb = consts.tile([1, BC], F32, tag="outsb")
        for nb in range(NBLK):
            pt = psum.tile([128, NB], F32)
            nc.tensor.matmul(pt[:1], rwsb[:], cat[:, nb * NB:(nb + 1) * NB],
                             start=True, stop=True)
            nc.vector.tensor_scalar_add(outsb[:1, nb * NB:(nb + 1) * NB],
                                        pt[:1], rbsb[:1])
        nc.sync.dma_start(out_d[:], outsb[:1, :])


_NC_CACHE = None


def _get_program():
    global _NC_CACHE
    if _NC_CACHE is None:
        _NC_CACHE = _build_program()
    return _NC_CACHE


def _prepare_in_maps(inputs):
    x = np.asarray(inputs["x"], np.float32)
    w1d = _densify(inputs["sl1_w"], inputs["sl1_out"], inputs["sl1_in"],
                   inputs["fc1_w"], L1, L2)
    w2d = _densify(inputs["sl2_w"], inputs["sl2_out"], inputs["sl2_in"],
                   inputs["fc2_w"], L2, L3)
    w3d = _densify(inputs["sl3_w"], inputs["sl3_out"], inputs["sl3_in"],
                   inputs["fc3_w"], L3, L4)
    w1p, w1fc, _ = _pack_w(w1d, L1, L2)
    w2p, w2fc, _ = _pack_w(w2d, L2, L3)
    w3p, w3fc, _ = _pack_w(w3d, L3, L4)
    shared = {
        "w1p": w1p, "w1fc": w1fc,
        "b1": _pack_b(inputs["sl1_b"], inputs["fc1_b"], L2),
        "w2p": w2p, "w2fc": w2fc,
        "b2": _pack_b(inputs["sl2_b"], inputs["fc2_b"], L3),
        "w3p": w3p, "w3fc": w3fc,
        "b3": _pack_b(inputs["sl3_b"], inputs["fc3_b"], L4),
        "w4": round_fp32r(np.asarray(inputs["fc4_w"], np.float32)
                          .reshape(4, 128).T.copy()),
        "fc4b": np.asarray(inputs["fc4_b"], np.float32).reshape(1, 1),
        "rw": round_fp32r(np.asarray(inputs["ro_w"], np.float32)
                          .reshape(4, 1).copy()),
        "rb": np.asarray(inputs["ro_b"], np.float32).reshape(1, 1),
    }
    in_maps = []
    for c in range(NCORES):
        xt = round_fp32r(
            np.ascontiguousarray(x[c * BC:(c + 1) * BC, :].T))
        in_maps.append({"xt": xt, **shared})
    return in_maps


def run(inputs, **kw):
    nc = _get_program()
    in_maps = _prepare_in_maps(inputs)
    res = bass_utils.run_bass_kernel_spmd(
        nc, in_maps, core_ids=list(range(NCORES)), **kw)
    out = np.concatenate([res.results[c]["out"].reshape(BC)
                          for c in range(NCORES)])
    return out.reshape(B, 1), res


def kernel(**inputs) -> np.ndarray:
    out, _ = run(inputs)
    return out



# revision 2
# speedup vs baseline: 1.2834x; 1.2834x over previous
"""Trainium2 Bass kernel for the HNN sparse-MLP network.

Strategy: densify the sparse edge lists into dense matrices on the host
and run the network as dense matmuls on the tensor engine, data-parallel
over the batch across 8 NeuronCores (1024 rows each).

Activations live feature-on-partition ([features, batch]) throughout:
    h_out[fo, b] = relu( sum_fi W[fi, fo] . h_in[fi, b] + bias )

Precision plan (rel tol 2e-2, measured ~2.5e-3):
  - layer-1 main [4096->2048]: bf16 (error here feeds the fc2 tap through
    a 2048-long dot product - fp8 would eat most of the error budget)
  - layer-2/3 mains: fp8 e4m3 with DoubleRow perf mode (2 K-planes per
    pass = 2x bf16 throughput); inputs h1,h2 are stored as scaled fp8
    copies written by a second activation pass per output tile
  - fc taps f1..f4 + readout: bf16 from bf16 activations (taps are long
    dot products whose error hits the output directly)

h1/h2/h3 stay SBUF-resident (no DRAM spill); weights stream per m-tile
with double buffering.
"""

import sys

sys.path.insert(0, "/opt/trn_rl_repo")

import numpy as np
import ml_dtypes

import concourse.bass as bass
import concourse.tile as tile
import concourse.mybir as mybir
from concourse import bacc, bass_utils

F32 = mybir.dt.float32
BF16 = mybir.dt.bfloat16
F8 = mybir.dt.float8e4
RELU = mybir.ActivationFunctionType.Relu
DR = mybir.MatmulPerfMode.DoubleRow

NP_BF16 = ml_dtypes.bfloat16
NP_F8 = ml_dtypes.float8_e4m3

NCORES = 8
B, L1, L2, L3, L4 = 8192, 4096, 2048, 1024, 512
BC = B // NCORES          # batch rows per core
NB = 512                  # matmul moving free dim (PSUM bank = 512 fp32)
NBLK = BC // NB

# fp8 scales (powers of two; descale folds into the activation)
S1 = 16.0                 # h1 fp8 storage scale (h1 max ~1.0)
S2 = 64.0                 # h2 fp8 storage scale (h2 max ~0.16)
SW2 = 64.0                # w2 fp8 scale
SW3 = 64.0                # w3 fp8 scale
D2 = 1.0 / (S1 * SW2)     # layer-2 psum descale
D3 = 1.0 / (S2 * SW3)     # layer-3 psum descale


def _densify(w, out_idx, in_idx, in_dim, out_dim):
    wd = np.zeros((in_dim, out_dim), np.float32)
    np.add.at(wd, (np.asarray(in_idx), np.asarray(out_idx)),
              np.asarray(w, np.float32))
    return wd


def _pack_w(wd, in_dim, out_dim):
    """[in_dim, out_dim] -> [T, 128, in_dim] with
    wp[t, p, j*128+m] = wd[j*128+p, t*128+m]."""
    kt, t = in_dim // 128, out_dim // 128
    return np.ascontiguousarray(
        wd.reshape(kt, 128, t, 128).transpose(2, 1, 0, 3).reshape(t, 128, in_dim))


def _to_f8(a, s):
    a = np.asarray(a, np.float32) * s
    assert np.abs(a).max() < 224.0, "fp8 overflow risk"
    return a.astype(NP_F8)


def _pack_b(b, out_dim):
    t = out_dim // 128
    return np.ascontiguousarray(np.asarray(b, np.float32).reshape(t, 128).T)


def _pack_fc(w, in_dim):
    """fc row [1, in_dim] -> [128, kt] bf16 (k-tile per column)."""
    kt = in_dim // 128
    return np.ascontiguousarray(
        np.asarray(w, np.float32).reshape(kt, 128).T).astype(NP_BF16)


def _build_program():
    nc = bacc.Bacc("TRN2", target_bir_lowering=False, debug=False,
                   num_devices=NCORES)
    d = {}
    d["xb"] = nc.dram_tensor("xb", [32, 128, BC], BF16, kind="ExternalInput").ap()
    d["w1p"] = nc.dram_tensor("w1p", [16, 128, L1], BF16, kind="ExternalInput").ap()
    d["w2p"] = nc.dram_tensor("w2p", [8, 128, L2], F8, kind="ExternalInput").ap()
    d["w3p"] = nc.dram_tensor("w3p", [4, 128, L3], F8, kind="ExternalInput").ap()
    d["f1w"] = nc.dram_tensor("f1w", [128, 32], BF16, kind="ExternalInput").ap()
    d["f2w"] = nc.dram_tensor("f2w", [128, 16], BF16, kind="ExternalInput").ap()
    d["f3w"] = nc.dram_tensor("f3w", [128, 8], BF16, kind="ExternalInput").ap()
    d["f4w"] = nc.dram_tensor("f4w", [128, 4], BF16, kind="ExternalInput").ap()
    d["b1"] = nc.dram_tensor("b1", [128, 16], F32, kind="ExternalInput").ap()
    d["b1s"] = nc.dram_tensor("b1s", [128, 16], F32, kind="ExternalInput").ap()
    d["b2"] = nc.dram_tensor("b2", [128, 8], F32, kind="ExternalInput").ap()
    d["b2s"] = nc.dram_tensor("b2s", [128, 8], F32, kind="ExternalInput").ap()
    d["b3"] = nc.dram_tensor("b3", [128, 4], F32, kind="ExternalInput").ap()
    for i in range(1, 5):
        d[f"fb{i}"] = nc.dram_tensor(f"fb{i}", [1, 1], F32, kind="ExternalInput").ap()
    d["rw"] = nc.dram_tensor("rw", [4, 1], BF16, kind="ExternalInput").ap()
    d["rb"] = nc.dram_tensor("rb", [1, 1], F32, kind="ExternalInput").ap()
    out_d = nc.dram_tensor("out", [1, BC], F32, kind="ExternalOutput").ap()

    with tile.TileContext(nc) as tc:
        _emit(nc, tc, d, out_d)
    nc.compile()
    return nc


def _emit(nc, tc, d, out_d):
    from contextlib import ExitStack

    with ExitStack() as ctx:
        consts = ctx.enter_context(tc.tile_pool(name="consts", bufs=1))
        psum = ctx.enter_context(tc.tile_pool(name="psum", bufs=4, space="PSUM"))

        def cload(name, shape, dt):
            t = consts.tile(shape, dt, tag=name)
            nc.sync.dma_start(t[:], d[name][:])
            return t

        # persistent activations + preloaded small weights
        h1b = consts.tile([128, 16, BC], BF16, tag="h1b")
        h18 = consts.tile([128, 16, BC], F8, tag="h18")
        h2b = consts.tile([128, 8, BC], BF16, tag="h2b")
        h28 = consts.tile([128, 8, BC], F8, tag="h28")
        h3b = consts.tile([128, 4, BC], BF16, tag="h3b")
        f1t = consts.tile([1, BC], BF16, tag="f1t")
        f2t = consts.tile([1, BC], BF16, tag="f2t")
        f3t = consts.tile([1, BC], BF16, tag="f3t")
        f4t = consts.tile([1, BC], BF16, tag="f4t")
        cat = consts.tile([4, BC], BF16, tag="cat")
        outsb = consts.tile([1, BC], F32, tag="outsb")

        with tc.tile_pool(name="xp", bufs=1) as xpool, \
             tc.tile_pool(name="w1m", bufs=3) as w1pool:
            # x load first: the L1 m=0 k-loop chases these arrivals
            xb = xpool.tile([128, 32, BC], BF16, tag="xb")
            xview = d["xb"]
            for j in range(32):
                nc.sync.dma_start(xb[:, j, :], xview[j])

            b1sb = cload("b1", [128, 16], F32)
            b1ss = cload("b1s", [128, 16], F32)
            b2sb = cload("b2", [128, 8], F32)
            b2ss = cload("b2s", [128, 8], F32)
            b3sb = cload("b3", [128, 4], F32)
            f1w = cload("f1w", [128, 32], BF16)
            f2w = cload("f2w", [128, 16], BF16)
            f3w = cload("f3w", [128, 8], BF16)
            f4w = cload("f4w", [128, 4], BF16)
            fb = [cload(f"fb{i}", [1, 1], F32) for i in range(1, 5)]
            rwsb = cload("rw", [4, 1], BF16)
            rbsb = cload("rb", [1, 1], F32)
            w2sb = consts.tile([128, 8, L2], F8, tag="w2sb")
            for m in range(8):
                nc.sync.dma_start(w2sb[:, m, :], d["w2p"][m])
            w3sb = consts.tile([128, 4, L3], F8, tag="w3sb")
            for m in range(4):
                nc.sync.dma_start(w3sb[:, m, :], d["w3p"][m])

            # ---- layer 1 main (bf16): x [4096,BC] -> h1 [2048,BC] ----
            for m in range(16):
                wt = w1pool.tile([128, L1], BF16, tag="w1m")
                nc.sync.dma_start(wt[:], d["w1p"][m])
                for nb in range(NBLK):
                    s = slice(nb * NB, (nb + 1) * NB)
                    pt = psum.tile([128, NB], F32)
                    for k in range(32):
                        nc.tensor.matmul(pt[:], wt[:, k * 128:(k + 1) * 128],
                                         xb[:, k, s],
                                         start=(k == 0), stop=(k == 31))
                    nc.scalar.activation(h1b[:, m, s], pt[:], RELU,
                                         bias=b1sb[:, m:m + 1])
                    nc.scalar.activation(h18[:, m, s], pt[:], RELU,
                                         bias=b1ss[:, m:m + 1], scale=S1)

            # ---- fc1 tap (bf16): f1 = relu(fc1 . x) ----
            for nb in range(NBLK):
                s = slice(nb * NB, (nb + 1) * NB)
                pt = psum.tile([128, NB], F32)
                for k in range(32):
                    nc.tensor.matmul(pt[:1], f1w[:, k:k + 1], xb[:, k, s],
                                     start=(k == 0), stop=(k == 31))
                nc.scalar.activation(f1t[:1, s], pt[:1], RELU, bias=fb[0][:1])

        # ---- fc2 tap (bf16 from h1b) ----
        for nb in range(NBLK):
            s = slice(nb * NB, (nb + 1) * NB)
            pt = psum.tile([128, NB], F32)
            for k in range(16):
                nc.tensor.matmul(pt[:1], f2w[:, k:k + 1], h1b[:, k, s],
                                 start=(k == 0), stop=(k == 15))
            nc.scalar.activation(f2t[:1, s], pt[:1], RELU, bias=fb[1][:1])

        # ---- layer 2 main (fp8 DoubleRow): h1 [2048,BC] -> h2 [1024,BC] ----
        h18v = h18.rearrange("p (a b) n -> p a b n", b=2)
        w2v = w2sb.rearrange("p m (a b c) -> p m a b c", b=2, c=128)
        for m in range(8):
            for nb in range(NBLK):
                s = slice(nb * NB, (nb + 1) * NB)
                pt = psum.tile([128, NB], F32)
                for kp in range(8):
                    nc.tensor.matmul(pt[:], w2v[:, m, kp], h18v[:, kp, :, s],
                                     start=(kp == 0), stop=(kp == 7),
                                     perf_mode=DR)
                nc.scalar.activation(h2b[:, m, s], pt[:], RELU,
                                     bias=b2sb[:, m:m + 1], scale=D2)
                nc.scalar.activation(h28[:, m, s], pt[:], RELU,
                                     bias=b2ss[:, m:m + 1], scale=D2 * S2)

        # ---- fc3 tap (bf16 from h2b) ----
        for nb in range(NBLK):
            s = slice(nb * NB, (nb + 1) * NB)
            pt = psum.tile([128, NB], F32)
            for k in range(8):
                nc.tensor.matmul(pt[:1], f3w[:, k:k + 1], h2b[:, k, s],
                                 start=(k == 0), stop=(k == 7))
            nc.scalar.activation(f3t[:1, s], pt[:1], RELU, bias=fb[2][:1])

        # ---- layer 3 main (fp8 DoubleRow): h2 [1024,BC] -> h3 [512,BC] ----
        h28v = h28.rearrange("p (a b) n -> p a b n", b=2)
        w3v = w3sb.rearrange("p m (a b c) -> p m a b c", b=2, c=128)
        for m in range(4):
            for nb in range(NBLK):
                s = slice(nb * NB, (nb + 1) * NB)
                pt = psum.tile([128, NB], F32)
                for kp in range(4):
                    nc.tensor.matmul(pt[:], w3v[:, m, kp], h28v[:, kp, :, s],
                                     start=(kp == 0), stop=(kp == 3),
                                     perf_mode=DR)
                nc.scalar.activation(h3b[:, m, s], pt[:], RELU,
                                     bias=b3sb[:, m:m + 1], scale=D3)

        # ---- fc4 tap (bf16 from h3b) ----
        for nb in range(NBLK):
            s = slice(nb * NB, (nb + 1) * NB)
            pt = psum.tile([128, NB], F32)
            for k in range(4):
                nc.tensor.matmul(pt[:1], f4w[:, k:k + 1], h3b[:, k, s],
                                 start=(k == 0), stop=(k == 3))
            nc.scalar.activation(f4t[:1, s], pt[:1], RELU, bias=fb[3][:1])

        # ---- readout ----
        nc.sync.dma_start(cat[0:1, :], f1t[0:1, :])
        nc.sync.dma_start(cat[1:2, :], f2t[0:1, :])
        nc.sync.dma_start(cat[2:3, :], f3t[0:1, :])
        nc.sync.dma_start(cat[3:4, :], f4t[0:1, :])
        for nb in range(NBLK):
            s = slice(nb * NB, (nb + 1) * NB)
            pt = psum.tile([128, NB], F32)
            nc.tensor.matmul(pt[:1], rwsb[:], cat[:, s], start=True, stop=True)
            nc.vector.tensor_scalar_add(outsb[:1, s], pt[:1], rbsb[:1])
        nc.sync.dma_start(out_d[:], outsb[:1, :])


_NC_CACHE = None


def _get_program():
    global _NC_CACHE
    if _NC_CACHE is None:
        _NC_CACHE = _build_program()
    return _NC_CACHE


def _prepare_in_maps(inputs):
    x = np.asarray(inputs["x"], np.float32)
    w1d = _densify(inputs["sl1_w"], inputs["sl1_out"], inputs["sl1_in"], L1, L2)
    w2d = _densify(inputs["sl2_w"], inputs["sl2_out"], inputs["sl2_in"], L2, L3)
    w3d = _densify(inputs["sl3_w"], inputs["sl3_out"], inputs["sl3_in"], L3, L4)
    b1 = _pack_b(inputs["sl1_b"], L2)
    b2 = _pack_b(inputs["sl2_b"], L3)
    shared = {
        "w1p": _pack_w(w1d, L1, L2).astype(NP_BF16),
        "w2p": _to_f8(_pack_w(w2d, L2, L3), SW2),
        "w3p": _to_f8(_pack_w(w3d, L3, L4), SW3),
        "f1w": _pack_fc(inputs["fc1_w"], L1),
        "f2w": _pack_fc(inputs["fc2_w"], L2),
        "f3w": _pack_fc(inputs["fc3_w"], L3),
        "f4w": _pack_fc(inputs["fc4_w"], L4),
        "b1": b1, "b1s": b1 * S1,
        "b2": b2, "b2s": b2 * S2,
        "b3": _pack_b(inputs["sl3_b"], L4),
        "fb1": np.asarray(inputs["fc1_b"], np.float32).reshape(1, 1),
        "fb2": np.asarray(inputs["fc2_b"], np.float32).reshape(1, 1),
        "fb3": np.asarray(inputs["fc3_b"], np.float32).reshape(1, 1),
        "fb4": np.asarray(inputs["fc4_b"], np.float32).reshape(1, 1),
        "rw": np.asarray(inputs["ro_w"], np.float32).reshape(4, 1).astype(NP_BF16),
        "rb": np.asarray(inputs["ro_b"], np.float32).reshape(1, 1),
    }
    in_maps = []
    for c in range(NCORES):
        xt = np.ascontiguousarray(x[c * BC:(c + 1) * BC, :].T)
        xb = np.ascontiguousarray(xt.reshape(32, 128, BC)).astype(NP_BF16)
        in_maps.append({"xb": xb, **shared})
    return in_maps


def run(inputs, **kw):
    nc = _get_program()
    in_maps = _prepare_in_maps(inputs)
    res = bass_utils.run_bass_kernel_spmd(
        nc, in_maps, core_ids=list(range(NCORES)), **kw)
    out = np.concatenate([res.results[c]["out"].reshape(BC)
                          for c in range(NCORES)])
    return out.reshape(B, 1), res


def kernel(**inputs) -> np.ndarray:
    out, _ = run(inputs)
    return out


# revision 4
# speedup vs baseline: 1.3877x; 1.0813x over previous
"""Trainium2 Bass kernel for the HNN sparse-MLP network.

Strategy: densify the sparse edge lists into dense matrices on the host
and run the network as dense matmuls on the tensor engine, data-parallel
over the batch across 8 NeuronCores (1024 rows each).

Activations live feature-on-partition ([features, batch]) throughout:
    h_out[fo, b] = relu( sum_fi W[fi, fo] . h_in[fi, b] + bias )

Precision plan (rel tol 2e-2, measured ~2.5e-3):
  - layer-1 main [4096->2048]: bf16 (error here feeds the fc2 tap through
    a 2048-long dot product - fp8 would eat most of the error budget)
  - layer-2/3 mains: fp8 e4m3 with DoubleRow perf mode (2 K-planes per
    pass = 2x bf16 throughput); inputs h1,h2 are stored as scaled fp8
    copies written by a second activation pass per output tile
  - fc taps f1..f4 + readout: bf16 from bf16 activations (taps are long
    dot products whose error hits the output directly)

h1/h2/h3 stay SBUF-resident (no DRAM spill); weights stream per m-tile
with double buffering.
"""

import sys

sys.path.insert(0, "/opt/trn_rl_repo")

import numpy as np
import ml_dtypes

import concourse.bass as bass
import concourse.tile as tile
import concourse.mybir as mybir
from concourse import bacc, bass_utils

F32 = mybir.dt.float32
BF16 = mybir.dt.bfloat16
F8 = mybir.dt.float8e4
RELU = mybir.ActivationFunctionType.Relu
DR = mybir.MatmulPerfMode.DoubleRow

NP_BF16 = ml_dtypes.bfloat16
NP_F8 = ml_dtypes.float8_e4m3

NCORES = 8
B, L1, L2, L3, L4 = 8192, 4096, 2048, 1024, 512
BC = B // NCORES          # batch rows per core
NB = 512                  # matmul moving free dim (PSUM bank = 512 fp32)
NBLK = BC // NB

# fp8 scales (powers of two; descale folds into the activation)
S1 = 16.0                 # h1 fp8 storage scale (h1 max ~1.0)
S2 = 64.0                 # h2 fp8 storage scale (h2 max ~0.16)
SW2 = 64.0                # w2 fp8 scale
SW3 = 64.0                # w3 fp8 scale
D2 = 1.0 / (S1 * SW2)     # layer-2 psum descale
D3 = 1.0 / (S2 * SW3)     # layer-3 psum descale


def _densify(w, out_idx, in_idx, in_dim, out_dim):
    wd = np.zeros((in_dim, out_dim), np.float32)
    np.add.at(wd, (np.asarray(in_idx), np.asarray(out_idx)),
              np.asarray(w, np.float32))
    return wd


def _pack_w(wd, in_dim, out_dim):
    """[in_dim, out_dim] -> [T, 128, in_dim] with
    wp[t, p, j*128+m] = wd[j*128+p, t*128+m]."""
    kt, t = in_dim // 128, out_dim // 128
    return np.ascontiguousarray(
        wd.reshape(kt, 128, t, 128).transpose(2, 1, 0, 3).reshape(t, 128, in_dim))


def _to_f8(a, s):
    a = np.asarray(a, np.float32) * s
    assert np.abs(a).max() < 224.0, "fp8 overflow risk"
    return a.astype(NP_F8)


def _pack_b(b, out_dim):
    t = out_dim // 128
    return np.ascontiguousarray(np.asarray(b, np.float32).reshape(t, 128).T)


def _pack_fc(w, in_dim):
    """fc row [1, in_dim] -> [128, kt] bf16 (k-tile per column)."""
    kt = in_dim // 128
    return np.ascontiguousarray(
        np.asarray(w, np.float32).reshape(kt, 128).T).astype(NP_BF16)


def _build_program():
    nc = bacc.Bacc("TRN2", target_bir_lowering=False, debug=False,
                   num_devices=NCORES)
    d = {}
    d["xb"] = nc.dram_tensor("xb", [32, 128, BC], BF16, kind="ExternalInput").ap()
    d["w1p"] = nc.dram_tensor("w1p", [16, 128, L1], BF16, kind="ExternalInput").ap()
    d["w2p"] = nc.dram_tensor("w2p", [8, 128, L2], F8, kind="ExternalInput").ap()
    d["w3p"] = nc.dram_tensor("w3p", [4, 128, L3], F8, kind="ExternalInput").ap()
    d["f1w"] = nc.dram_tensor("f1w", [128, 32], BF16, kind="ExternalInput").ap()
    d["f2w"] = nc.dram_tensor("f2w", [128, 16], BF16, kind="ExternalInput").ap()
    d["f3w"] = nc.dram_tensor("f3w", [128, 8], BF16, kind="ExternalInput").ap()
    d["f4w"] = nc.dram_tensor("f4w", [128, 4], BF16, kind="ExternalInput").ap()
    d["b1"] = nc.dram_tensor("b1", [128, 16], F32, kind="ExternalInput").ap()
    d["b1s"] = nc.dram_tensor("b1s", [128, 16], F32, kind="ExternalInput").ap()
    d["b2"] = nc.dram_tensor("b2", [128, 8], F32, kind="ExternalInput").ap()
    d["b2s"] = nc.dram_tensor("b2s", [128, 8], F32, kind="ExternalInput").ap()
    d["b3"] = nc.dram_tensor("b3", [128, 4], F32, kind="ExternalInput").ap()
    for i in range(1, 5):
        d[f"fb{i}"] = nc.dram_tensor(f"fb{i}", [1, 1], F32, kind="ExternalInput").ap()
    d["rw"] = nc.dram_tensor("rw", [4, 1], BF16, kind="ExternalInput").ap()
    d["rb"] = nc.dram_tensor("rb", [1, 1], F32, kind="ExternalInput").ap()
    out_d = nc.dram_tensor("out", [1, BC], F32, kind="ExternalOutput").ap()

    with tile.TileContext(nc) as tc:
        _emit(nc, tc, d, out_d)
    nc.compile()
    return nc


def _emit(nc, tc, d, out_d):
    from contextlib import ExitStack

    with ExitStack() as ctx:
        consts = ctx.enter_context(tc.tile_pool(name="consts", bufs=1))
        psum = ctx.enter_context(tc.tile_pool(name="psum", bufs=4, space="PSUM"))

        def cload(name, shape, dt):
            t = consts.tile(shape, dt, tag=name)
            nc.sync.dma_start(t[:], d[name][:])
            return t

        # persistent activations + preloaded small weights
        h1b = consts.tile([128, 16, BC], BF16, tag="h1b")
        h18 = consts.tile([128, 16, BC], F8, tag="h18")
        h2b = consts.tile([128, 8, BC], BF16, tag="h2b")
        h28 = consts.tile([128, 8, BC], F8, tag="h28")
        h3b = consts.tile([128, 4, BC], BF16, tag="h3b")
        f1t = consts.tile([1, BC], BF16, tag="f1t")
        f2t = consts.tile([1, BC], BF16, tag="f2t")
        f3t = consts.tile([1, BC], BF16, tag="f3t")
        f4t = consts.tile([1, BC], BF16, tag="f4t")
        cat = consts.tile([4, BC], BF16, tag="cat")
        outsb = consts.tile([1, BC], F32, tag="outsb")

        with tc.tile_pool(name="xp", bufs=1) as xpool, \
             tc.tile_pool(name="w1m", bufs=3) as w1pool:
            # DMA priority: m=0 weights first (gates the first matmul), then
            # x tiles split across both DGE engines, then small consts.
            # w2/w3 are loaded after the L1 loop - they aren't needed until
            # ~200us in and would delay the critical path here.
            wts = {}

            def w1_load(m):
                wt = w1pool.tile([128, L1], BF16, tag="w1m")
                nc.sync.dma_start(wt[:], d["w1p"][m])
                wts[m] = wt

            w1_load(0)
            xb = xpool.tile([128, 32, BC], BF16, tag="xb")
            xview = d["xb"]
            for j in range(32):
                eng = nc.sync if j % 2 == 0 else nc.scalar
                eng.dma_start(xb[:, j, :], xview[j])
            w1_load(1)

            b1sb = cload("b1", [128, 16], F32)
            b1ss = cload("b1s", [128, 16], F32)
            b2sb = cload("b2", [128, 8], F32)
            b2ss = cload("b2s", [128, 8], F32)
            b3sb = cload("b3", [128, 4], F32)
            f1w = cload("f1w", [128, 32], BF16)
            f2w = cload("f2w", [128, 16], BF16)
            f3w = cload("f3w", [128, 8], BF16)
            f4w = cload("f4w", [128, 4], BF16)
            fb = [cload(f"fb{i}", [1, 1], F32) for i in range(1, 5)]
            rwsb = cload("rw", [4, 1], BF16)
            rbsb = cload("rb", [1, 1], F32)

            # ---- layer 1 main (bf16): x [4096,BC] -> h1 [2048,BC] ----
            for m in range(16):
                if m not in wts:
                    w1_load(m)
                wt = wts.pop(m)
                if m + 2 <= 15 and m + 2 not in wts:
                    w1_load(m + 2)
                for nb in range(NBLK):
                    s = slice(nb * NB, (nb + 1) * NB)
                    pt = psum.tile([128, NB], F32)
                    for k in range(32):
                        nc.tensor.matmul(pt[:], wt[:, k * 128:(k + 1) * 128],
                                         xb[:, k, s],
                                         start=(k == 0), stop=(k == 31))
                    nc.scalar.activation(h1b[:, m, s], pt[:], RELU,
                                         bias=b1sb[:, m:m + 1])
                    nc.scalar.activation(h18[:, m, s], pt[:], RELU,
                                         bias=b1ss[:, m:m + 1], scale=S1)

            # ---- fc1 tap (bf16): f1 = relu(fc1 . x) ----
            for nb in range(NBLK):
                s = slice(nb * NB, (nb + 1) * NB)
                pt = psum.tile([128, NB], F32)
                for k in range(32):
                    nc.tensor.matmul(pt[:1], f1w[:, k:k + 1], xb[:, k, s],
                                     start=(k == 0), stop=(k == 31))
                nc.scalar.activation(f1t[:1, s], pt[:1], RELU, bias=fb[0][:1])

        # layer-2/3 weights: small, needed only after L1 - load late so they
        # don't contend with the startup-critical x/w1 transfers
        w2sb = consts.tile([128, 8, L2], F8, tag="w2sb")
        for m in range(8):
            nc.sync.dma_start(w2sb[:, m, :], d["w2p"][m])
        w3sb = consts.tile([128, 4, L3], F8, tag="w3sb")
        for m in range(4):
            nc.sync.dma_start(w3sb[:, m, :], d["w3p"][m])

        # ---- fc2 tap (bf16 from h1b) ----
        for nb in range(NBLK):
            s = slice(nb * NB, (nb + 1) * NB)
            pt = psum.tile([128, NB], F32)
            for k in range(16):
                nc.tensor.matmul(pt[:1], f2w[:, k:k + 1], h1b[:, k, s],
                                 start=(k == 0), stop=(k == 15))
            nc.scalar.activation(f2t[:1, s], pt[:1], RELU, bias=fb[1][:1])

        # ---- layer 2 main (fp8 DoubleRow): h1 [2048,BC] -> h2 [1024,BC] ----
        h18v = h18.rearrange("p (a b) n -> p a b n", b=2)
        w2v = w2sb.rearrange("p m (a b c) -> p m a b c", b=2, c=128)
        for m in range(8):
            for nb in range(NBLK):
                s = slice(nb * NB, (nb + 1) * NB)
                pt = psum.tile([128, NB], F32)
                for kp in range(8):
                    nc.tensor.matmul(pt[:], w2v[:, m, kp], h18v[:, kp, :, s],
                                     start=(kp == 0), stop=(kp == 7),
                                     perf_mode=DR)
                nc.scalar.activation(h2b[:, m, s], pt[:], RELU,
                                     bias=b2sb[:, m:m + 1], scale=D2)
                nc.scalar.activation(h28[:, m, s], pt[:], RELU,
                                     bias=b2ss[:, m:m + 1], scale=D2 * S2)

        # ---- fc3 tap (bf16 from h2b) ----
        for nb in range(NBLK):
            s = slice(nb * NB, (nb + 1) * NB)
            pt = psum.tile([128, NB], F32)
            for k in range(8):
                nc.tensor.matmul(pt[:1], f3w[:, k:k + 1], h2b[:, k, s],
                                 start=(k == 0), stop=(k == 7))
            nc.scalar.activation(f3t[:1, s], pt[:1], RELU, bias=fb[2][:1])

        # ---- layer 3 main (fp8 DoubleRow): h2 [1024,BC] -> h3 [512,BC] ----
        h28v = h28.rearrange("p (a b) n -> p a b n", b=2)
        w3v = w3sb.rearrange("p m (a b c) -> p m a b c", b=2, c=128)
        for m in range(4):
            for nb in range(NBLK):
                s = slice(nb * NB, (nb + 1) * NB)
                pt = psum.tile([128, NB], F32)
                for kp in range(4):
                    nc.tensor.matmul(pt[:], w3v[:, m, kp], h28v[:, kp, :, s],
                                     start=(kp == 0), stop=(kp == 3),
                                     perf_mode=DR)
                nc.scalar.activation(h3b[:, m, s], pt[:], RELU,
                                     bias=b3sb[:, m:m + 1], scale=D3)

        # ---- fc4 tap (bf16 from h3b) ----
        for nb in range(NBLK):
            s = slice(nb * NB, (nb + 1) * NB)
            pt = psum.tile([128, NB], F32)
            for k in range(4):
                nc.tensor.matmul(pt[:1], f4w[:, k:k + 1], h3b[:, k, s],
                                 start=(k == 0), stop=(k == 3))
            nc.scalar.activation(f4t[:1, s], pt[:1], RELU, bias=fb[3][:1])

        # ---- readout ----
        nc.sync.dma_start(cat[0:1, :], f1t[0:1, :])
        nc.sync.dma_start(cat[1:2, :], f2t[0:1, :])
        nc.sync.dma_start(cat[2:3, :], f3t[0:1, :])
        nc.sync.dma_start(cat[3:4, :], f4t[0:1, :])
        for nb in range(NBLK):
            s = slice(nb * NB, (nb + 1) * NB)
            pt = psum.tile([128, NB], F32)
            nc.tensor.matmul(pt[:1], rwsb[:], cat[:, s], start=True, stop=True)
            nc.vector.tensor_scalar_add(outsb[:1, s], pt[:1], rbsb[:1])
        nc.sync.dma_start(out_d[:], outsb[:1, :])


_NC_CACHE = None


def _get_program():
    global _NC_CACHE
    if _NC_CACHE is None:
        _NC_CACHE = _build_program()
    return _NC_CACHE


def _prepare_in_maps(inputs):
    x = np.asarray(inputs["x"], np.float32)
    w1d = _densify(inputs["sl1_w"], inputs["sl1_out"], inputs["sl1_in"], L1, L2)
    w2d = _densify(inputs["sl2_w"], inputs["sl2_out"], inputs["sl2_in"], L2, L3)
    w3d = _densify(inputs["sl3_w"], inputs["sl3_out"], inputs["sl3_in"], L3, L4)
    b1 = _pack_b(inputs["sl1_b"], L2)
    b2 = _pack_b(inputs["sl2_b"], L3)
    shared = {
        "w1p": _pack_w(w1d, L1, L2).astype(NP_BF16),
        "w2p": _to_f8(_pack_w(w2d, L2, L3), SW2),
        "w3p": _to_f8(_pack_w(w3d, L3, L4), SW3),
        "f1w": _pack_fc(inputs["fc1_w"], L1),
        "f2w": _pack_fc(inputs["fc2_w"], L2),
        "f3w": _pack_fc(inputs["fc3_w"], L3),
        "f4w": _pack_fc(inputs["fc4_w"], L4),
        "b1": b1, "b1s": b1 * S1,
        "b2": b2, "b2s": b2 * S2,
        "b3": _pack_b(inputs["sl3_b"], L4),
        "fb1": np.asarray(inputs["fc1_b"], np.float32).reshape(1, 1),
        "fb2": np.asarray(inputs["fc2_b"], np.float32).reshape(1, 1),
        "fb3": np.asarray(inputs["fc3_b"], np.float32).reshape(1, 1),
        "fb4": np.asarray(inputs["fc4_b"], np.float32).reshape(1, 1),
        "rw": np.asarray(inputs["ro_w"], np.float32).reshape(4, 1).astype(NP_BF16),
        "rb": np.asarray(inputs["ro_b"], np.float32).reshape(1, 1),
    }
    in_maps = []
    for c in range(NCORES):
        xt = np.ascontiguousarray(x[c * BC:(c + 1) * BC, :].T)
        xb = np.ascontiguousarray(xt.reshape(32, 128, BC)).astype(NP_BF16)
        in_maps.append({"xb": xb, **shared})
    return in_maps


def run(inputs, **kw):
    nc = _get_program()
    in_maps = _prepare_in_maps(inputs)
    res = bass_utils.run_bass_kernel_spmd(
        nc, in_maps, core_ids=list(range(NCORES)), **kw)
    out = np.concatenate([res.results[c]["out"].reshape(BC)
                          for c in range(NCORES)])
    return out.reshape(B, 1), res


def kernel(**inputs) -> np.ndarray:
    out, _ = run(inputs)
    return out


# revision 13
# speedup vs baseline: 1.9489x; 1.4043x over previous
"""Trainium2 Bass kernel for the HNN sparse-MLP network.

Strategy: densify the sparse edge lists into dense matrices on the host
and run the network as dense matmuls on the tensor engine, data-parallel
over the batch across 8 NeuronCores (1024 rows each).

Activations live feature-on-partition ([features, batch]) throughout:
    h_out[fo, b] = relu( sum_fi W[fi, fo] . h_in[fi, b] + bias )

Precision plan (rel tol 2e-2, measured ~2.5e-3):
  - layer-1 main [4096->2048]: bf16 (error here feeds the fc2 tap through
    a 2048-long dot product - fp8 would eat most of the error budget)
  - layer-2/3 mains: fp8 e4m3 with DoubleRow perf mode (2 K-planes per
    pass = 2x bf16 throughput); inputs h1,h2 are stored as scaled fp8
    copies written by a second activation pass per output tile
  - fc taps f1..f4 + readout: bf16 from bf16 activations (taps are long
    dot products whose error hits the output directly)

h1/h2/h3 stay SBUF-resident (no DRAM spill); weights stream per m-tile
with double buffering.
"""

import sys

sys.path.insert(0, "/opt/trn_rl_repo")

import numpy as np
import ml_dtypes

import concourse.bass as bass
import concourse.tile as tile
import concourse.mybir as mybir
from concourse import bacc, bass_utils

F32 = mybir.dt.float32
BF16 = mybir.dt.bfloat16
F8 = mybir.dt.float8e4
RELU = mybir.ActivationFunctionType.Relu
DR = mybir.MatmulPerfMode.DoubleRow

NP_BF16 = ml_dtypes.bfloat16
NP_F8 = ml_dtypes.float8_e4m3

NCORES = 8
B, L1, L2, L3, L4 = 8192, 4096, 2048, 1024, 512
BC = B // NCORES          # batch rows per core
NB = 512                  # matmul moving free dim (PSUM bank = 512 fp32)
NBLK = BC // NB

# fp8 scales (powers of two; descale folds into the activation)
S1 = 16.0                 # h1 fp8 storage scale (h1 max ~1.0)
S2 = 64.0                 # h2 fp8 storage scale (h2 max ~0.16)
SW2 = 64.0                # w2 fp8 scale
SW3 = 64.0                # w3 fp8 scale
D2 = 1.0 / (S1 * SW2)     # layer-2 psum descale
D3 = 1.0 / (S2 * SW3)     # layer-3 psum descale

# layer-1 K-split: first KF8 of 32 k-tiles run fp8 e4m3 DoubleRow (2x),
# the rest bf16. The bf16 weights are pre-scaled by SX*SW1 (exact pow2)
# so both parts share one psum accumulation group and one descale.
KF8 = 32
SX = 8.0                  # x fp8 scale
SW1 = 64.0                # w1 fp8 scale
D1 = 1.0 / (SX * SW1)     # layer-1 psum descale


def _densify(w, out_idx, in_idx, in_dim, out_dim):
    wd = np.zeros((in_dim, out_dim), np.float32)
    np.add.at(wd, (np.asarray(in_idx), np.asarray(out_idx)),
              np.asarray(w, np.float32))
    return wd


def _pack_w(wd, in_dim, out_dim):
    """[in_dim, out_dim] -> [T, 128, in_dim] with
    wp[t, p, j*128+m] = wd[j*128+p, t*128+m]."""
    kt, t = in_dim // 128, out_dim // 128
    return np.ascontiguousarray(
        wd.reshape(kt, 128, t, 128).transpose(2, 1, 0, 3).reshape(t, 128, in_dim))


def _to_f8(a, s):
    a = np.asarray(a, np.float32) * s
    assert np.abs(a).max() < 224.0, "fp8 overflow risk"
    return a.astype(NP_F8)


def _pack_b(b, out_dim):
    t = out_dim // 128
    return np.ascontiguousarray(np.asarray(b, np.float32).reshape(t, 128).T)


def _pack_fc(w, in_dim):
    """fc row [1, in_dim] -> [128, kt] bf16 (k-tile per column)."""
    kt = in_dim // 128
    return np.ascontiguousarray(
        np.asarray(w, np.float32).reshape(kt, 128).T).astype(NP_BF16)


def _build_program():
    nc = bacc.Bacc("TRN2", target_bir_lowering=False, debug=False,
                   num_devices=NCORES)
    d = {}
    d["xb"] = nc.dram_tensor("xb", [32, 128, BC], BF16, kind="ExternalInput").ap()
    if KF8 > 0:
        d["x8"] = nc.dram_tensor("x8", [KF8, 128, BC], F8, kind="ExternalInput").ap()
        d["w1p8"] = nc.dram_tensor("w1p8", [16, 128, KF8 * 128], F8,
                                   kind="ExternalInput").ap()
    if KF8 < 32:
        d["w1pb"] = nc.dram_tensor("w1pb", [16, 128, (32 - KF8) * 128], BF16,
                                   kind="ExternalInput").ap()
    d["w2p"] = nc.dram_tensor("w2p", [8, 128, L2], F8, kind="ExternalInput").ap()
    d["w3p"] = nc.dram_tensor("w3p", [4, 128, L3], F8, kind="ExternalInput").ap()
    d["f1w"] = nc.dram_tensor("f1w", [128, 32], BF16, kind="ExternalInput").ap()
    d["f2w"] = nc.dram_tensor("f2w", [128, 16], BF16, kind="ExternalInput").ap()
    d["f3w"] = nc.dram_tensor("f3w", [128, 8], BF16, kind="ExternalInput").ap()
    d["f4w"] = nc.dram_tensor("f4w", [128, 4], BF16, kind="ExternalInput").ap()
    d["b1"] = nc.dram_tensor("b1", [128, 16], F32, kind="ExternalInput").ap()
    d["b1s"] = nc.dram_tensor("b1s", [128, 16], F32, kind="ExternalInput").ap()
    d["b2"] = nc.dram_tensor("b2", [128, 8], F32, kind="ExternalInput").ap()
    d["b2s"] = nc.dram_tensor("b2s", [128, 8], F32, kind="ExternalInput").ap()
    d["b3"] = nc.dram_tensor("b3", [128, 4], F32, kind="ExternalInput").ap()
    for i in range(1, 5):
        d[f"fb{i}"] = nc.dram_tensor(f"fb{i}", [1, 1], F32, kind="ExternalInput").ap()
    d["rw"] = nc.dram_tensor("rw", [4, 1], BF16, kind="ExternalInput").ap()
    d["rb"] = nc.dram_tensor("rb", [1, 1], F32, kind="ExternalInput").ap()
    out_d = nc.dram_tensor("out", [1, BC], F32, kind="ExternalOutput").ap()

    with tile.TileContext(nc) as tc:
        _emit(nc, tc, d, out_d)
    nc.compile()
    return nc


def _emit(nc, tc, d, out_d):
    from contextlib import ExitStack

    with ExitStack() as ctx:
        consts = ctx.enter_context(tc.tile_pool(name="consts", bufs=1))
        psum = ctx.enter_context(tc.tile_pool(name="psum", bufs=4, space="PSUM"))

        def cload(name, shape, dt):
            t = consts.tile(shape, dt, tag=name)
            nc.sync.dma_start(t[:], d[name][:])
            return t

        # persistent activations + preloaded small weights
        h1b = consts.tile([128, 16, BC], BF16, tag="h1b")
        h18 = consts.tile([128, 16, BC], F8, tag="h18")
        h2b = consts.tile([128, 8, BC], BF16, tag="h2b")
        h28 = consts.tile([128, 8, BC], F8, tag="h28")
        h3b = consts.tile([128, 4, BC], BF16, tag="h3b")
        f1t = consts.tile([1, BC], BF16, tag="f1t")
        f2t = consts.tile([1, BC], BF16, tag="f2t")
        f3t = consts.tile([1, BC], BF16, tag="f3t")
        f4t = consts.tile([1, BC], BF16, tag="f4t")
        cat = consts.tile([4, BC], BF16, tag="cat")
        outsb = consts.tile([1, BC], F32, tag="outsb")

        with tc.tile_pool(name="xp", bufs=1) as xpool, \
             tc.tile_pool(name="w1m", bufs=3) as w1pool:
            # DMA priority: m=0 weights first (gates the first matmul), then
            # x tiles split across both DGE engines, then small consts.
            # w2/w3 are loaded after the L1 loop - they aren't needed until
            # late and would delay the critical path here.
            wts = {}

            def w1_load(m):
                parts = []
                if KF8 > 0:
                    w8 = w1pool.tile([128, KF8 * 128], F8, tag="w1m8")
                    nc.sync.dma_start(w8[:], d["w1p8"][m])
                    parts.append(w8)
                else:
                    parts.append(None)
                if KF8 < 32:
                    wb = w1pool.tile([128, (32 - KF8) * 128], BF16, tag="w1mb")
                    nc.scalar.dma_start(wb[:], d["w1pb"][m])
                    parts.append(wb)
                else:
                    parts.append(None)
                wts[m] = parts

            w1_load(0)
            xb = xpool.tile([128, 32, BC], BF16, tag="xb")
            xview = d["xb"]
            if KF8 > 0:
                x8 = xpool.tile([128, KF8, BC], F8, tag="x8")
                for j in range(KF8):
                    eng = nc.sync if j % 2 == 0 else nc.scalar
                    eng.dma_start(x8[:, j, :], d["x8"][j])
                x8v = x8.rearrange("p (a b) n -> p a b n", b=2)
            for j in range(32):
                eng = nc.sync if j % 2 == 0 else nc.scalar
                eng.dma_start(xb[:, j, :], xview[j])
            w1_load(1)

            b1sb = cload("b1", [128, 16], F32)
            b1ss = cload("b1s", [128, 16], F32)
            b2sb = cload("b2", [128, 8], F32)
            b2ss = cload("b2s", [128, 8], F32)
            b3sb = cload("b3", [128, 4], F32)
            f1w = cload("f1w", [128, 32], BF16)
            f2w = cload("f2w", [128, 16], BF16)
            f3w = cload("f3w", [128, 8], BF16)
            f4w = cload("f4w", [128, 4], BF16)
            fb = [cload(f"fb{i}", [1, 1], F32) for i in range(1, 5)]
            rwsb = cload("rw", [4, 1], BF16)
            rbsb = cload("rb", [1, 1], F32)

            # ---- layer 1 main: x [4096,BC] -> h1 [2048,BC] ----
            # k-tiles [0,KF8) fp8 DoubleRow, [KF8,32) bf16, one psum group
            for m in range(16):
                if m not in wts:
                    w1_load(m)
                w8t, wbt = wts.pop(m)
                if m + 2 <= 15 and m + 2 not in wts:
                    w1_load(m + 2)
                if w8t is not None:
                    w8v = w8t.rearrange("p (a b c) -> p a b c", b=2, c=128)
                for nb in range(NBLK):
                    s = slice(nb * NB, (nb + 1) * NB)
                    pt = psum.tile([128, NB], F32)
                    for kp in range(KF8 // 2):
                        nc.tensor.matmul(pt[:], w8v[:, kp], x8v[:, kp, :, s],
                                         start=(kp == 0),
                                         stop=(KF8 == 32 and kp == 15),
                                         perf_mode=DR)
                    for k in range(KF8, 32):
                        nc.tensor.matmul(
                            pt[:], wbt[:, (k - KF8) * 128:(k - KF8 + 1) * 128],
                            xb[:, k, s],
                            start=(k == KF8 == 0), stop=(k == 31))
                    nc.scalar.activation(h1b[:, m, s], pt[:], RELU,
                                         bias=b1sb[:, m:m + 1], scale=D1)
                    nc.scalar.activation(h18[:, m, s], pt[:], RELU,
                                         bias=b1ss[:, m:m + 1], scale=D1 * S1)

            # ---- fc1 tap (bf16): f1 = relu(fc1 . x) ----
            for nb in range(NBLK):
                s = slice(nb * NB, (nb + 1) * NB)
                pt = psum.tile([128, NB], F32)
                for k in range(32):
                    nc.tensor.matmul(pt[:1], f1w[:, k:k + 1], xb[:, k, s],
                                     start=(k == 0), stop=(k == 31))
                nc.scalar.activation(f1t[:1, s], pt[:1], RELU, bias=fb[0][:1])
            nc.sync.dma_start(cat[0:1, :], f1t[0:1, :])

        # layer-2/3 weights: small, needed only after L1 - loaded late (no
        # contention with the startup-critical x/w1 transfers) and allocated
        # after the x pool closes so they reuse its SBUF space
        l23 = ctx.enter_context(tc.tile_pool(name="l23w", bufs=1))
        w2sb = l23.tile([128, 8, L2], F8, tag="w2sb")
        for m in range(8):
            nc.sync.dma_start(w2sb[:, m, :], d["w2p"][m])
        w3sb = l23.tile([128, 4, L3], F8, tag="w3sb")
        for m in range(4):
            nc.sync.dma_start(w3sb[:, m, :], d["w3p"][m])

        # ---- fc2 tap (bf16 from h1b) ----
        for nb in range(NBLK):
            s = slice(nb * NB, (nb + 1) * NB)
            pt = psum.tile([128, NB], F32)
            for k in range(16):
                nc.tensor.matmul(pt[:1], f2w[:, k:k + 1], h1b[:, k, s],
                                 start=(k == 0), stop=(k == 15))
            nc.scalar.activation(f2t[:1, s], pt[:1], RELU, bias=fb[1][:1])
        nc.sync.dma_start(cat[1:2, :], f2t[0:1, :])

        # ---- layer 2 main (fp8 DoubleRow): h1 [2048,BC] -> h2 [1024,BC] ----
        h18v = h18.rearrange("p (a b) n -> p a b n", b=2)
        w2v = w2sb.rearrange("p m (a b c) -> p m a b c", b=2, c=128)
        for m in range(8):
            for nb in range(NBLK):
                s = slice(nb * NB, (nb + 1) * NB)
                pt = psum.tile([128, NB], F32)
                for kp in range(8):
                    nc.tensor.matmul(pt[:], w2v[:, m, kp], h18v[:, kp, :, s],
                                     start=(kp == 0), stop=(kp == 7),
                                     perf_mode=DR)
                nc.scalar.activation(h2b[:, m, s], pt[:], RELU,
                                     bias=b2sb[:, m:m + 1], scale=D2)
                nc.scalar.activation(h28[:, m, s], pt[:], RELU,
                                     bias=b2ss[:, m:m + 1], scale=D2 * S2)

        # ---- fc3 tap (bf16 from h2b) ----
        for nb in range(NBLK):
            s = slice(nb * NB, (nb + 1) * NB)
            pt = psum.tile([128, NB], F32)
            for k in range(8):
                nc.tensor.matmul(pt[:1], f3w[:, k:k + 1], h2b[:, k, s],
                                 start=(k == 0), stop=(k == 7))
            nc.scalar.activation(f3t[:1, s], pt[:1], RELU, bias=fb[2][:1])
        nc.sync.dma_start(cat[2:3, :], f3t[0:1, :])

        # ---- layer 3 main (fp8 DoubleRow): h2 [1024,BC] -> h3 [512,BC] ----
        h28v = h28.rearrange("p (a b) n -> p a b n", b=2)
        w3v = w3sb.rearrange("p m (a b c) -> p m a b c", b=2, c=128)
        for m in range(4):
            for nb in range(NBLK):
                s = slice(nb * NB, (nb + 1) * NB)
                pt = psum.tile([128, NB], F32)
                for kp in range(4):
                    nc.tensor.matmul(pt[:], w3v[:, m, kp], h28v[:, kp, :, s],
                                     start=(kp == 0), stop=(kp == 3),
                                     perf_mode=DR)
                nc.scalar.activation(h3b[:, m, s], pt[:], RELU,
                                     bias=b3sb[:, m:m + 1], scale=D3)

        # ---- fc4 tap (bf16 from h3b) ----
        for nb in range(NBLK):
            s = slice(nb * NB, (nb + 1) * NB)
            pt = psum.tile([128, NB], F32)
            for k in range(4):
                nc.tensor.matmul(pt[:1], f4w[:, k:k + 1], h3b[:, k, s],
                                 start=(k == 0), stop=(k == 3))
            nc.scalar.activation(f4t[:1, s], pt[:1], RELU, bias=fb[3][:1])

        # ---- readout ----
        nc.sync.dma_start(cat[3:4, :], f4t[0:1, :])
        for nb in range(NBLK):
            s = slice(nb * NB, (nb + 1) * NB)
            pt = psum.tile([128, NB], F32)
            nc.tensor.matmul(pt[:1], rwsb[:], cat[:, s], start=True, stop=True)
            nc.vector.tensor_scalar_add(outsb[:1, s], pt[:1], rbsb[:1])
        nc.sync.dma_start(out_d[:], outsb[:1, :])


_NC_CACHE = None


def _get_program():
    global _NC_CACHE
    if _NC_CACHE is None:
        _NC_CACHE = _build_program()
    return _NC_CACHE


def _prepare_in_maps(inputs):
    x = np.asarray(inputs["x"], np.float32)
    w1d = _densify(inputs["sl1_w"], inputs["sl1_out"], inputs["sl1_in"], L1, L2)
    w2d = _densify(inputs["sl2_w"], inputs["sl2_out"], inputs["sl2_in"], L2, L3)
    w3d = _densify(inputs["sl3_w"], inputs["sl3_out"], inputs["sl3_in"], L3, L4)
    b1 = _pack_b(inputs["sl1_b"], L2)
    b2 = _pack_b(inputs["sl2_b"], L3)
    w1pk = _pack_w(w1d, L1, L2)
    shared = {
        "w2p": _to_f8(_pack_w(w2d, L2, L3), SW2),
        "w3p": _to_f8(_pack_w(w3d, L3, L4), SW3),
        "f1w": _pack_fc(inputs["fc1_w"], L1),
        "f2w": _pack_fc(inputs["fc2_w"], L2),
        "f3w": _pack_fc(inputs["fc3_w"], L3),
        "f4w": _pack_fc(inputs["fc4_w"], L4),
        "b1": b1, "b1s": b1 * S1,
        "b2": b2, "b2s": b2 * S2,
        "b3": _pack_b(inputs["sl3_b"], L4),
        "fb1": np.asarray(inputs["fc1_b"], np.float32).reshape(1, 1),
        "fb2": np.asarray(inputs["fc2_b"], np.float32).reshape(1, 1),
        "fb3": np.asarray(inputs["fc3_b"], np.float32).reshape(1, 1),
        "fb4": np.asarray(inputs["fc4_b"], np.float32).reshape(1, 1),
        "rw": np.asarray(inputs["ro_w"], np.float32).reshape(4, 1).astype(NP_BF16),
        "rb": np.asarray(inputs["ro_b"], np.float32).reshape(1, 1),
    }
    if KF8 > 0:
        shared["w1p8"] = _to_f8(w1pk[:, :, :KF8 * 128], SW1)
    if KF8 < 32:
        shared["w1pb"] = np.ascontiguousarray(
            w1pk[:, :, KF8 * 128:] * (SX * SW1)).astype(NP_BF16)
    in_maps = []
    for c in range(NCORES):
        xt = np.ascontiguousarray(x[c * BC:(c + 1) * BC, :].T)
        xtr = np.ascontiguousarray(xt.reshape(32, 128, BC))
        per = {"xb": xtr.astype(NP_BF16), **shared}
        if KF8 > 0:
            per["x8"] = _to_f8(xtr[:KF8], SX)
        in_maps.append(per)
    return in_maps


def run(inputs, **kw):
    nc = _get_program()
    in_maps = _prepare_in_maps(inputs)
    res = bass_utils.run_bass_kernel_spmd(
        nc, in_maps, core_ids=list(range(NCORES)), **kw)
    out = np.concatenate([res.results[c]["out"].reshape(BC)
                          for c in range(NCORES)])
    return out.reshape(B, 1), res


def kernel(**inputs) -> np.ndarray:
    out, _ = run(inputs)
    return out


# revision 15
# speedup vs baseline: 2.1152x; 1.0853x over previous
"""Trainium2 Bass kernel for the HNN sparse-MLP network.

Strategy: densify the sparse edge lists into dense matrices on the host
and run the network as dense matmuls on the tensor engine, data-parallel
over the batch across 8 NeuronCores (1024 rows each).

Activations live feature-on-partition ([features, batch]) throughout:
    h_out[fo, b] = relu( sum_fi W[fi, fo] . h_in[fi, b] + bias )

Precision plan (rel tol 2e-2, measured ~2.5e-3):
  - layer-1 main [4096->2048]: bf16 (error here feeds the fc2 tap through
    a 2048-long dot product - fp8 would eat most of the error budget)
  - layer-2/3 mains: fp8 e4m3 with DoubleRow perf mode (2 K-planes per
    pass = 2x bf16 throughput); inputs h1,h2 are stored as scaled fp8
    copies written by a second activation pass per output tile
  - fc taps f1..f4 + readout: bf16 from bf16 activations (taps are long
    dot products whose error hits the output directly)

h1/h2/h3 stay SBUF-resident (no DRAM spill); weights stream per m-tile
with double buffering.
"""

import sys

sys.path.insert(0, "/opt/trn_rl_repo")

import numpy as np
import ml_dtypes

import concourse.bass as bass
import concourse.tile as tile
import concourse.mybir as mybir
from concourse import bacc, bass_utils

F32 = mybir.dt.float32
BF16 = mybir.dt.bfloat16
F8 = mybir.dt.float8e4
RELU = mybir.ActivationFunctionType.Relu
DR = mybir.MatmulPerfMode.DoubleRow

NP_BF16 = ml_dtypes.bfloat16
NP_F8 = ml_dtypes.float8_e4m3

NCORES = 8
B, L1, L2, L3, L4 = 8192, 4096, 2048, 1024, 512
BC = B // NCORES          # batch rows per core
NB = 512                  # matmul moving free dim (PSUM bank = 512 fp32)
NBLK = BC // NB

# fp8 scales (powers of two; descale folds into the activation)
S1 = 16.0                 # h1 fp8 storage scale (h1 max ~1.0)
S2 = 64.0                 # h2 fp8 storage scale (h2 max ~0.16)
SW2 = 64.0                # w2 fp8 scale
SW3 = 64.0                # w3 fp8 scale
D2 = 1.0 / (S1 * SW2)     # layer-2 psum descale
D3 = 1.0 / (S2 * SW3)     # layer-3 psum descale

# layer-1 K-split: first KF8 of 32 k-tiles run fp8 e4m3 DoubleRow (2x),
# the rest bf16. The bf16 weights are pre-scaled by SX*SW1 (exact pow2)
# so both parts share one psum accumulation group and one descale.
KF8 = 32
SX = 8.0                  # x fp8 scale
SW1 = 64.0                # w1 fp8 scale
D1 = 1.0 / (SX * SW1)     # layer-1 psum descale


def _densify(w, out_idx, in_idx, in_dim, out_dim):
    wd = np.zeros((in_dim, out_dim), np.float32)
    np.add.at(wd, (np.asarray(in_idx), np.asarray(out_idx)),
              np.asarray(w, np.float32))
    return wd


def _pack_w(wd, in_dim, out_dim):
    """[in_dim, out_dim] -> [T, 128, in_dim] with
    wp[t, p, j*128+m] = wd[j*128+p, t*128+m]."""
    kt, t = in_dim // 128, out_dim // 128
    return np.ascontiguousarray(
        wd.reshape(kt, 128, t, 128).transpose(2, 1, 0, 3).reshape(t, 128, in_dim))


def _to_f8(a, s):
    a = np.asarray(a, np.float32) * s
    assert np.abs(a).max() < 224.0, "fp8 overflow risk"
    return a.astype(NP_F8)


def _pack_b(b, out_dim):
    t = out_dim // 128
    return np.ascontiguousarray(np.asarray(b, np.float32).reshape(t, 128).T)


def _pack_fc(w, in_dim):
    """fc row [1, in_dim] -> [128, kt] bf16 (k-tile per column)."""
    kt = in_dim // 128
    return np.ascontiguousarray(
        np.asarray(w, np.float32).reshape(kt, 128).T).astype(NP_BF16)


def _build_program():
    nc = bacc.Bacc("TRN2", target_bir_lowering=False, debug=False,
                   num_devices=NCORES)
    d = {}
    d["xb"] = nc.dram_tensor("xb", [32, 128, BC], BF16, kind="ExternalInput").ap()
    if KF8 > 0:
        d["x8"] = nc.dram_tensor("x8", [KF8, 128, BC], F8, kind="ExternalInput").ap()
        d["w1p8"] = nc.dram_tensor("w1p8", [16, 128, KF8 * 128], F8,
                                   kind="ExternalInput").ap()
    if KF8 < 32:
        d["w1pb"] = nc.dram_tensor("w1pb", [16, 128, (32 - KF8) * 128], BF16,
                                   kind="ExternalInput").ap()
    d["w2p"] = nc.dram_tensor("w2p", [8, 128, L2], F8, kind="ExternalInput").ap()
    d["w3p"] = nc.dram_tensor("w3p", [4, 128, L3], F8, kind="ExternalInput").ap()
    d["f1w"] = nc.dram_tensor("f1w", [128, 32], BF16, kind="ExternalInput").ap()
    d["f2w"] = nc.dram_tensor("f2w", [128, 16], BF16, kind="ExternalInput").ap()
    d["f3w"] = nc.dram_tensor("f3w", [128, 8], BF16, kind="ExternalInput").ap()
    d["f4w"] = nc.dram_tensor("f4w", [128, 4], BF16, kind="ExternalInput").ap()
    d["b1"] = nc.dram_tensor("b1", [128, 16], F32, kind="ExternalInput").ap()
    d["b1s"] = nc.dram_tensor("b1s", [128, 16], F32, kind="ExternalInput").ap()
    d["b2"] = nc.dram_tensor("b2", [128, 8], F32, kind="ExternalInput").ap()
    d["b2s"] = nc.dram_tensor("b2s", [128, 8], F32, kind="ExternalInput").ap()
    d["b3"] = nc.dram_tensor("b3", [128, 4], F32, kind="ExternalInput").ap()
    for i in range(1, 5):
        d[f"fb{i}"] = nc.dram_tensor(f"fb{i}", [1, 1], F32, kind="ExternalInput").ap()
    d["rw"] = nc.dram_tensor("rw", [4, 1], BF16, kind="ExternalInput").ap()
    d["rb"] = nc.dram_tensor("rb", [1, 1], F32, kind="ExternalInput").ap()
    out_d = nc.dram_tensor("out", [1, BC], F32, kind="ExternalOutput").ap()

    with tile.TileContext(nc) as tc:
        _emit(nc, tc, d, out_d)
    nc.compile()
    return nc


def _emit(nc, tc, d, out_d):
    from contextlib import ExitStack

    with ExitStack() as ctx:
        consts = ctx.enter_context(tc.tile_pool(name="consts", bufs=1))
        psum = ctx.enter_context(tc.tile_pool(name="psum", bufs=4, space="PSUM"))

        def cload(name, shape, dt):
            t = consts.tile(shape, dt, tag=name)
            nc.sync.dma_start(t[:], d[name][:])
            return t

        # persistent activations + preloaded small weights
        h1b = consts.tile([128, 16, BC], BF16, tag="h1b")
        h18 = consts.tile([128, 16, BC], F8, tag="h18")
        h2b = consts.tile([128, 8, BC], BF16, tag="h2b")
        h28 = consts.tile([128, 8, BC], F8, tag="h28")
        h3b = consts.tile([128, 4, BC], BF16, tag="h3b")
        f1t = consts.tile([1, BC], BF16, tag="f1t")
        f2t = consts.tile([1, BC], BF16, tag="f2t")
        f3t = consts.tile([1, BC], BF16, tag="f3t")
        f4t = consts.tile([1, BC], BF16, tag="f4t")
        cat = consts.tile([4, BC], BF16, tag="cat")
        outsb = consts.tile([1, BC], F32, tag="outsb")

        with tc.tile_pool(name="xp", bufs=1) as xpool, \
             tc.tile_pool(name="w1m", bufs=3) as w1pool:
            # DMA priority: m=0 weights first (gates the first matmul), then
            # x tiles split across both DGE engines, then small consts.
            # w2/w3 are loaded after the L1 loop - they aren't needed until
            # late and would delay the critical path here.
            wts = {}

            def w1_load(m):
                parts = []
                if KF8 > 0:
                    w8 = w1pool.tile([128, KF8 * 128], F8, tag="w1m8")
                    nc.sync.dma_start(w8[:], d["w1p8"][m])
                    parts.append(w8)
                else:
                    parts.append(None)
                if KF8 < 32:
                    wb = w1pool.tile([128, (32 - KF8) * 128], BF16, tag="w1mb")
                    nc.scalar.dma_start(wb[:], d["w1pb"][m])
                    parts.append(wb)
                else:
                    parts.append(None)
                wts[m] = parts

            w1_load(0)
            xb = xpool.tile([128, 32, BC], BF16, tag="xb")
            xview = d["xb"]
            if KF8 > 0:
                x8 = xpool.tile([128, KF8, BC], F8, tag="x8")
                for j in range(KF8):
                    eng = nc.sync if j % 2 == 0 else nc.scalar
                    eng.dma_start(x8[:, j, :], d["x8"][j])
                x8v = x8.rearrange("p (a b) n -> p a b n", b=2)

            def xb_load():
                for j in range(32):
                    eng = nc.sync if j % 2 == 0 else nc.scalar
                    eng.dma_start(xb[:, j, :], xview[j])

            if KF8 < 32:
                # bf16 part of L1 reads xb from the first m-tile on
                xb_load()
            w1_load(1)

            b1sb = cload("b1", [128, 16], F32)
            b1ss = cload("b1s", [128, 16], F32)
            b2sb = cload("b2", [128, 8], F32)
            b2ss = cload("b2s", [128, 8], F32)
            b3sb = cload("b3", [128, 4], F32)
            f1w = cload("f1w", [128, 32], BF16)
            f2w = cload("f2w", [128, 16], BF16)
            f3w = cload("f3w", [128, 8], BF16)
            f4w = cload("f4w", [128, 4], BF16)
            fb = [cload(f"fb{i}", [1, 1], F32) for i in range(1, 5)]
            rwsb = cload("rw", [4, 1], BF16)
            rbsb = cload("rb", [1, 1], F32)

            # ---- layer 1 main: x [4096,BC] -> h1 [2048,BC] ----
            # k-tiles [0,KF8) fp8 DoubleRow, [KF8,32) bf16, one psum group
            for m in range(16):
                if m not in wts:
                    w1_load(m)
                w8t, wbt = wts.pop(m)
                if m + 2 <= 15 and m + 2 not in wts:
                    w1_load(m + 2)
                if w8t is not None:
                    w8v = w8t.rearrange("p (a b c) -> p a b c", b=2, c=128)
                for nb in range(NBLK):
                    s = slice(nb * NB, (nb + 1) * NB)
                    pt = psum.tile([128, NB], F32)
                    for kp in range(KF8 // 2):
                        nc.tensor.matmul(pt[:], w8v[:, kp], x8v[:, kp, :, s],
                                         start=(kp == 0),
                                         stop=(KF8 == 32 and kp == 15),
                                         perf_mode=DR)
                    for k in range(KF8, 32):
                        nc.tensor.matmul(
                            pt[:], wbt[:, (k - KF8) * 128:(k - KF8 + 1) * 128],
                            xb[:, k, s],
                            start=(k == KF8 == 0), stop=(k == 31))
                    nc.scalar.activation(h1b[:, m, s], pt[:], RELU,
                                         bias=b1sb[:, m:m + 1], scale=D1)
                    nc.scalar.activation(h18[:, m, s], pt[:], RELU,
                                         bias=b1ss[:, m:m + 1], scale=D1 * S1)

            if KF8 == 32:
                # xb only feeds the fc1 tap - load after the L1 weight
                # stream so it doesn't delay the critical path
                xb_load()

            # ---- fc1 tap (bf16): f1 = relu(fc1 . x) ----
            for nb in range(NBLK):
                s = slice(nb * NB, (nb + 1) * NB)
                pt = psum.tile([128, NB], F32)
                for k in range(32):
                    nc.tensor.matmul(pt[:1], f1w[:, k:k + 1], xb[:, k, s],
                                     start=(k == 0), stop=(k == 31))
                nc.scalar.activation(f1t[:1, s], pt[:1], RELU, bias=fb[0][:1])
            nc.sync.dma_start(cat[0:1, :], f1t[0:1, :])

        # layer-2/3 weights: small, needed only after L1 - loaded late (no
        # contention with the startup-critical x/w1 transfers) and allocated
        # after the x pool closes so they reuse its SBUF space
        l23 = ctx.enter_context(tc.tile_pool(name="l23w", bufs=1))
        w2sb = l23.tile([128, 8, L2], F8, tag="w2sb")
        for m in range(8):
            nc.sync.dma_start(w2sb[:, m, :], d["w2p"][m])
        w3sb = l23.tile([128, 4, L3], F8, tag="w3sb")
        for m in range(4):
            nc.sync.dma_start(w3sb[:, m, :], d["w3p"][m])

        # ---- fc2 tap (bf16 from h1b) ----
        for nb in range(NBLK):
            s = slice(nb * NB, (nb + 1) * NB)
            pt = psum.tile([128, NB], F32)
            for k in range(16):
                nc.tensor.matmul(pt[:1], f2w[:, k:k + 1], h1b[:, k, s],
                                 start=(k == 0), stop=(k == 15))
            nc.scalar.activation(f2t[:1, s], pt[:1], RELU, bias=fb[1][:1])
        nc.sync.dma_start(cat[1:2, :], f2t[0:1, :])

        # ---- layer 2 main (fp8 DoubleRow): h1 [2048,BC] -> h2 [1024,BC] ----
        h18v = h18.rearrange("p (a b) n -> p a b n", b=2)
        w2v = w2sb.rearrange("p m (a b c) -> p m a b c", b=2, c=128)
        for m in range(8):
            for nb in range(NBLK):
                s = slice(nb * NB, (nb + 1) * NB)
                pt = psum.tile([128, NB], F32)
                for kp in range(8):
                    nc.tensor.matmul(pt[:], w2v[:, m, kp], h18v[:, kp, :, s],
                                     start=(kp == 0), stop=(kp == 7),
                                     perf_mode=DR)
                nc.scalar.activation(h2b[:, m, s], pt[:], RELU,
                                     bias=b2sb[:, m:m + 1], scale=D2)
                nc.scalar.activation(h28[:, m, s], pt[:], RELU,
                                     bias=b2ss[:, m:m + 1], scale=D2 * S2)

        # ---- fc3 tap (bf16 from h2b) ----
        for nb in range(NBLK):
            s = slice(nb * NB, (nb + 1) * NB)
            pt = psum.tile([128, NB], F32)
            for k in range(8):
                nc.tensor.matmul(pt[:1], f3w[:, k:k + 1], h2b[:, k, s],
                                 start=(k == 0), stop=(k == 7))
            nc.scalar.activation(f3t[:1, s], pt[:1], RELU, bias=fb[2][:1])
        nc.sync.dma_start(cat[2:3, :], f3t[0:1, :])

        # ---- layer 3 main (fp8 DoubleRow): h2 [1024,BC] -> h3 [512,BC] ----
        h28v = h28.rearrange("p (a b) n -> p a b n", b=2)
        w3v = w3sb.rearrange("p m (a b c) -> p m a b c", b=2, c=128)
        for m in range(4):
            for nb in range(NBLK):
                s = slice(nb * NB, (nb + 1) * NB)
                pt = psum.tile([128, NB], F32)
                for kp in range(4):
                    nc.tensor.matmul(pt[:], w3v[:, m, kp], h28v[:, kp, :, s],
                                     start=(kp == 0), stop=(kp == 3),
                                     perf_mode=DR)
                nc.scalar.activation(h3b[:, m, s], pt[:], RELU,
                                     bias=b3sb[:, m:m + 1], scale=D3)

        # ---- fc4 tap (bf16 from h3b) ----
        for nb in range(NBLK):
            s = slice(nb * NB, (nb + 1) * NB)
            pt = psum.tile([128, NB], F32)
            for k in range(4):
                nc.tensor.matmul(pt[:1], f4w[:, k:k + 1], h3b[:, k, s],
                                 start=(k == 0), stop=(k == 3))
            nc.scalar.activation(f4t[:1, s], pt[:1], RELU, bias=fb[3][:1])

        # ---- readout ----
        nc.sync.dma_start(cat[3:4, :], f4t[0:1, :])
        for nb in range(NBLK):
            s = slice(nb * NB, (nb + 1) * NB)
            pt = psum.tile([128, NB], F32)
            nc.tensor.matmul(pt[:1], rwsb[:], cat[:, s], start=True, stop=True)
            nc.vector.tensor_scalar_add(outsb[:1, s], pt[:1], rbsb[:1])
        nc.sync.dma_start(out_d[:], outsb[:1, :])


_NC_CACHE = None


def _get_program():
    global _NC_CACHE
    if _NC_CACHE is None:
        _NC_CACHE = _build_program()
    return _NC_CACHE


def _prepare_in_maps(inputs):
    x = np.asarray(inputs["x"], np.float32)
    w1d = _densify(inputs["sl1_w"], inputs["sl1_out"], inputs["sl1_in"], L1, L2)
    w2d = _densify(inputs["sl2_w"], inputs["sl2_out"], inputs["sl2_in"], L2, L3)
    w3d = _densify(inputs["sl3_w"], inputs["sl3_out"], inputs["sl3_in"], L3, L4)
    b1 = _pack_b(inputs["sl1_b"], L2)
    b2 = _pack_b(inputs["sl2_b"], L3)
    w1pk = _pack_w(w1d, L1, L2)
    shared = {
        "w2p": _to_f8(_pack_w(w2d, L2, L3), SW2),
        "w3p": _to_f8(_pack_w(w3d, L3, L4), SW3),
        "f1w": _pack_fc(inputs["fc1_w"], L1),
        "f2w": _pack_fc(inputs["fc2_w"], L2),
        "f3w": _pack_fc(inputs["fc3_w"], L3),
        "f4w": _pack_fc(inputs["fc4_w"], L4),
        "b1": b1, "b1s": b1 * S1,
        "b2": b2, "b2s": b2 * S2,
        "b3": _pack_b(inputs["sl3_b"], L4),
        "fb1": np.asarray(inputs["fc1_b"], np.float32).reshape(1, 1),
        "fb2": np.asarray(inputs["fc2_b"], np.float32).reshape(1, 1),
        "fb3": np.asarray(inputs["fc3_b"], np.float32).reshape(1, 1),
        "fb4": np.asarray(inputs["fc4_b"], np.float32).reshape(1, 1),
        "rw": np.asarray(inputs["ro_w"], np.float32).reshape(4, 1).astype(NP_BF16),
        "rb": np.asarray(inputs["ro_b"], np.float32).reshape(1, 1),
    }
    if KF8 > 0:
        shared["w1p8"] = _to_f8(w1pk[:, :, :KF8 * 128], SW1)
    if KF8 < 32:
        shared["w1pb"] = np.ascontiguousarray(
            w1pk[:, :, KF8 * 128:] * (SX * SW1)).astype(NP_BF16)
    in_maps = []
    for c in range(NCORES):
        xt = np.ascontiguousarray(x[c * BC:(c + 1) * BC, :].T)
        xtr = np.ascontiguousarray(xt.reshape(32, 128, BC))
        per = {"xb": xtr.astype(NP_BF16), **shared}
        if KF8 > 0:
            per["x8"] = _to_f8(xtr[:KF8], SX)
        in_maps.append(per)
    return in_maps


def run(inputs, **kw):
    nc = _get_program()
    in_maps = _prepare_in_maps(inputs)
    res = bass_utils.run_bass_kernel_spmd(
        nc, in_maps, core_ids=list(range(NCORES)), **kw)
    out = np.concatenate([res.results[c]["out"].reshape(BC)
                          for c in range(NCORES)])
    return out.reshape(B, 1), res


def kernel(**inputs) -> np.ndarray:
    out, _ = run(inputs)
    return out


# revision 19
# speedup vs baseline: 2.1403x; 1.0119x over previous
"""Trainium2 Bass kernel for the HNN sparse-MLP network.

Strategy: densify the sparse edge lists into dense matrices on the host
and run the network as dense matmuls on the tensor engine, data-parallel
over the batch across 8 NeuronCores (1024 rows each).

Activations live feature-on-partition ([features, batch]) throughout:
    h_out[fo, b] = relu( sum_fi W[fi, fo] . h_in[fi, b] + bias )

Precision plan (rel tol 2e-2, measured ~2.5e-3):
  - layer-1 main [4096->2048]: bf16 (error here feeds the fc2 tap through
    a 2048-long dot product - fp8 would eat most of the error budget)
  - layer-2/3 mains: fp8 e4m3 with DoubleRow perf mode (2 K-planes per
    pass = 2x bf16 throughput); inputs h1,h2 are stored as scaled fp8
    copies written by a second activation pass per output tile
  - fc taps f1..f4 + readout: bf16 from bf16 activations (taps are long
    dot products whose error hits the output directly)

h1/h2/h3 stay SBUF-resident (no DRAM spill); weights stream per m-tile
with double buffering.
"""

import sys

sys.path.insert(0, "/opt/trn_rl_repo")

import numpy as np
import ml_dtypes

import concourse.bass as bass
import concourse.tile as tile
import concourse.mybir as mybir
from concourse import bacc, bass_utils

F32 = mybir.dt.float32
BF16 = mybir.dt.bfloat16
F8 = mybir.dt.float8e4
RELU = mybir.ActivationFunctionType.Relu
DR = mybir.MatmulPerfMode.DoubleRow

NP_BF16 = ml_dtypes.bfloat16
NP_F8 = ml_dtypes.float8_e4m3

NCORES = 8
B, L1, L2, L3, L4 = 8192, 4096, 2048, 1024, 512
BC = B // NCORES          # batch rows per core
NB = 512                  # matmul moving free dim (PSUM bank = 512 fp32)
NBLK = BC // NB

# fp8 scales (powers of two; descale folds into the activation)
S1 = 16.0                 # h1 fp8 storage scale (h1 max ~1.0)
S2 = 64.0                 # h2 fp8 storage scale (h2 max ~0.16)
SW2 = 64.0                # w2 fp8 scale
SW3 = 64.0                # w3 fp8 scale
D2 = 1.0 / (S1 * SW2)     # layer-2 psum descale
D3 = 1.0 / (S2 * SW3)     # layer-3 psum descale

# layer-1 K-split: first KF8 of 32 k-tiles run fp8 e4m3 DoubleRow (2x),
# the rest bf16. The bf16 weights are pre-scaled by SX*SW1 (exact pow2)
# so both parts share one psum accumulation group and one descale.
KF8 = 32
SX = 8.0                  # x fp8 scale
SW1 = 64.0                # w1 fp8 scale
D1 = 1.0 / (SX * SW1)     # layer-1 psum descale


def _densify(w, out_idx, in_idx, in_dim, out_dim):
    wd = np.zeros((in_dim, out_dim), np.float32)
    np.add.at(wd, (np.asarray(in_idx), np.asarray(out_idx)),
              np.asarray(w, np.float32))
    return wd


def _pack_w(wd, in_dim, out_dim):
    """[in_dim, out_dim] -> [T, 128, in_dim] with
    wp[t, p, j*128+m] = wd[j*128+p, t*128+m]."""
    kt, t = in_dim // 128, out_dim // 128
    return np.ascontiguousarray(
        wd.reshape(kt, 128, t, 128).transpose(2, 1, 0, 3).reshape(t, 128, in_dim))


def _to_f8(a, s):
    a = np.asarray(a, np.float32) * s
    assert np.abs(a).max() < 224.0, "fp8 overflow risk"
    return a.astype(NP_F8)


def _pack_b(b, out_dim):
    t = out_dim // 128
    return np.ascontiguousarray(np.asarray(b, np.float32).reshape(t, 128).T)


def _pack_fc(w, in_dim):
    """fc row [1, in_dim] -> [128, kt] bf16 (k-tile per column)."""
    kt = in_dim // 128
    return np.ascontiguousarray(
        np.asarray(w, np.float32).reshape(kt, 128).T).astype(NP_BF16)


def _build_program():
    nc = bacc.Bacc("TRN2", target_bir_lowering=False, debug=False,
                   num_devices=NCORES)
    d = {}
    d["xb"] = nc.dram_tensor("xb", [32, 128, BC], BF16, kind="ExternalInput").ap()
    if KF8 > 0:
        d["x8"] = nc.dram_tensor("x8", [KF8, 128, BC], F8, kind="ExternalInput").ap()
        d["w1p8"] = nc.dram_tensor("w1p8", [16, 128, KF8 * 128], F8,
                                   kind="ExternalInput").ap()
    if KF8 < 32:
        d["w1pb"] = nc.dram_tensor("w1pb", [16, 128, (32 - KF8) * 128], BF16,
                                   kind="ExternalInput").ap()
    d["w2p"] = nc.dram_tensor("w2p", [8, 128, L2], F8, kind="ExternalInput").ap()
    d["w3p"] = nc.dram_tensor("w3p", [4, 128, L3], F8, kind="ExternalInput").ap()
    d["f1w"] = nc.dram_tensor("f1w", [128, 32], BF16, kind="ExternalInput").ap()
    d["f2w"] = nc.dram_tensor("f2w", [128, 16], BF16, kind="ExternalInput").ap()
    d["f3w"] = nc.dram_tensor("f3w", [128, 8], BF16, kind="ExternalInput").ap()
    d["f4w"] = nc.dram_tensor("f4w", [128, 4], BF16, kind="ExternalInput").ap()
    d["b1"] = nc.dram_tensor("b1", [128, 16], F32, kind="ExternalInput").ap()
    d["b1s"] = nc.dram_tensor("b1s", [128, 16], F32, kind="ExternalInput").ap()
    d["b2"] = nc.dram_tensor("b2", [128, 8], F32, kind="ExternalInput").ap()
    d["b2s"] = nc.dram_tensor("b2s", [128, 8], F32, kind="ExternalInput").ap()
    d["b3"] = nc.dram_tensor("b3", [128, 4], F32, kind="ExternalInput").ap()
    for i in range(1, 5):
        d[f"fb{i}"] = nc.dram_tensor(f"fb{i}", [1, 1], F32, kind="ExternalInput").ap()
    d["rw"] = nc.dram_tensor("rw", [4, 1], BF16, kind="ExternalInput").ap()
    d["rb"] = nc.dram_tensor("rb", [1, 1], F32, kind="ExternalInput").ap()
    out_d = nc.dram_tensor("out", [1, BC], F32, kind="ExternalOutput").ap()

    with tile.TileContext(nc) as tc:
        _emit(nc, tc, d, out_d)
    nc.compile()
    return nc


def _emit(nc, tc, d, out_d):
    from contextlib import ExitStack

    with ExitStack() as ctx:
        consts = ctx.enter_context(tc.tile_pool(name="consts", bufs=1))
        psum = ctx.enter_context(tc.tile_pool(name="psum", bufs=4, space="PSUM"))

        def cload(name, shape, dt):
            t = consts.tile(shape, dt, tag=name)
            nc.sync.dma_start(t[:], d[name][:])
            return t

        # persistent activations + preloaded small weights
        h1b = consts.tile([128, 16, BC], BF16, tag="h1b")
        h18 = consts.tile([128, 16, BC], F8, tag="h18")
        h2b = consts.tile([128, 8, BC], BF16, tag="h2b")
        h28 = consts.tile([128, 8, BC], F8, tag="h28")
        h3b = consts.tile([128, 4, BC], BF16, tag="h3b")
        f1t = consts.tile([1, BC], BF16, tag="f1t")
        f2t = consts.tile([1, BC], BF16, tag="f2t")
        f3t = consts.tile([1, BC], BF16, tag="f3t")
        f4t = consts.tile([1, BC], BF16, tag="f4t")
        cat = consts.tile([4, BC], BF16, tag="cat")
        outsb = consts.tile([1, BC], F32, tag="outsb")

        with tc.tile_pool(name="xp", bufs=1) as xpool, \
             tc.tile_pool(name="w1m", bufs=4) as w1pool:
            # DMA priority: m=0 weights first (gates the first matmul), then
            # x tiles split across both DGE engines, then small consts.
            # w2/w3 are loaded after the L1 loop - they aren't needed until
            # late and would delay the critical path here.
            wts = {}

            def w1_load(m):
                parts = []
                if KF8 > 0:
                    w8 = w1pool.tile([128, KF8 * 128], F8, tag="w1m8")
                    half = KF8 * 64
                    nc.sync.dma_start(w8[:, :half], d["w1p8"][m, :, :half])
                    nc.scalar.dma_start(w8[:, half:], d["w1p8"][m, :, half:])
                    parts.append(w8)
                else:
                    parts.append(None)
                if KF8 < 32:
                    wb = w1pool.tile([128, (32 - KF8) * 128], BF16, tag="w1mb")
                    nc.scalar.dma_start(wb[:], d["w1pb"][m])
                    parts.append(wb)
                else:
                    parts.append(None)
                wts[m] = parts

            w1_load(0)
            xb = xpool.tile([128, 32, BC], BF16, tag="xb")
            xview = d["xb"]
            if KF8 > 0:
                x8 = xpool.tile([128, KF8, BC], F8, tag="x8")
                for j in range(KF8):
                    eng = nc.sync if j % 2 == 0 else nc.scalar
                    eng.dma_start(x8[:, j, :], d["x8"][j])
                x8v = x8.rearrange("p (a b) n -> p a b n", b=2)

            def xb_load():
                for j in range(32):
                    eng = nc.sync if j % 2 == 0 else nc.scalar
                    eng.dma_start(xb[:, j, :], xview[j])

            if KF8 < 32:
                # bf16 part of L1 reads xb from the first m-tile on
                xb_load()
            w1_load(1)

            b1sb = cload("b1", [128, 16], F32)
            b1ss = cload("b1s", [128, 16], F32)
            b2sb = cload("b2", [128, 8], F32)
            b2ss = cload("b2s", [128, 8], F32)
            b3sb = cload("b3", [128, 4], F32)
            f1w = cload("f1w", [128, 32], BF16)
            f2w = cload("f2w", [128, 16], BF16)
            f3w = cload("f3w", [128, 8], BF16)
            f4w = cload("f4w", [128, 4], BF16)
            fb = [cload(f"fb{i}", [1, 1], F32) for i in range(1, 5)]
            rwsb = cload("rw", [4, 1], BF16)
            rbsb = cload("rb", [1, 1], F32)
            w1_load(2)

            # ---- layer 1 main: x [4096,BC] -> h1 [2048,BC] ----
            # k-tiles [0,KF8) fp8 DoubleRow, [KF8,32) bf16, one psum group
            for m in range(16):
                if m not in wts:
                    w1_load(m)
                w8t, wbt = wts.pop(m)
                if m + 3 <= 15 and m + 3 not in wts:
                    w1_load(m + 3)
                if w8t is not None:
                    w8v = w8t.rearrange("p (a b c) -> p a b c", b=2, c=128)
                for nb in range(NBLK):
                    s = slice(nb * NB, (nb + 1) * NB)
                    pt = psum.tile([128, NB], F32)
                    for kp in range(KF8 // 2):
                        nc.tensor.matmul(pt[:], w8v[:, kp], x8v[:, kp, :, s],
                                         start=(kp == 0),
                                         stop=(KF8 == 32 and kp == 15),
                                         perf_mode=DR)
                    for k in range(KF8, 32):
                        nc.tensor.matmul(
                            pt[:], wbt[:, (k - KF8) * 128:(k - KF8 + 1) * 128],
                            xb[:, k, s],
                            start=(k == KF8 == 0), stop=(k == 31))
                    nc.scalar.activation(h1b[:, m, s], pt[:], RELU,
                                         bias=b1sb[:, m:m + 1], scale=D1)
                    nc.scalar.activation(h18[:, m, s], pt[:], RELU,
                                         bias=b1ss[:, m:m + 1], scale=D1 * S1)

            if KF8 == 32:
                # xb only feeds the fc1 tap - load after the L1 weight
                # stream so it doesn't delay the critical path
                xb_load()

            # ---- fc1 tap (bf16): f1 = relu(fc1 . x) ----
            for nb in range(NBLK):
                s = slice(nb * NB, (nb + 1) * NB)
                pt = psum.tile([128, NB], F32)
                for k in range(32):
                    nc.tensor.matmul(pt[:1], f1w[:, k:k + 1], xb[:, k, s],
                                     start=(k == 0), stop=(k == 31))
                nc.scalar.activation(f1t[:1, s], pt[:1], RELU, bias=fb[0][:1])
            nc.sync.dma_start(cat[0:1, :], f1t[0:1, :])

        # layer-2/3 weights: small, needed only after L1 - loaded late (no
        # contention with the startup-critical x/w1 transfers) and allocated
        # after the x pool closes so they reuse its SBUF space
        l23 = ctx.enter_context(tc.tile_pool(name="l23w", bufs=1))
        w2sb = l23.tile([128, 8, L2], F8, tag="w2sb")
        for m in range(8):
            nc.sync.dma_start(w2sb[:, m, :], d["w2p"][m])
        w3sb = l23.tile([128, 4, L3], F8, tag="w3sb")
        for m in range(4):
            nc.sync.dma_start(w3sb[:, m, :], d["w3p"][m])

        # ---- fc2 tap (bf16 from h1b) ----
        for nb in range(NBLK):
            s = slice(nb * NB, (nb + 1) * NB)
            pt = psum.tile([128, NB], F32)
            for k in range(16):
                nc.tensor.matmul(pt[:1], f2w[:, k:k + 1], h1b[:, k, s],
                                 start=(k == 0), stop=(k == 15))
            nc.scalar.activation(f2t[:1, s], pt[:1], RELU, bias=fb[1][:1])
        nc.sync.dma_start(cat[1:2, :], f2t[0:1, :])

        # ---- layer 2 main (fp8 DoubleRow): h1 [2048,BC] -> h2 [1024,BC] ----
        h18v = h18.rearrange("p (a b) n -> p a b n", b=2)
        w2v = w2sb.rearrange("p m (a b c) -> p m a b c", b=2, c=128)
        for m in range(8):
            for nb in range(NBLK):
                s = slice(nb * NB, (nb + 1) * NB)
                pt = psum.tile([128, NB], F32)
                for kp in range(8):
                    nc.tensor.matmul(pt[:], w2v[:, m, kp], h18v[:, kp, :, s],
                                     start=(kp == 0), stop=(kp == 7),
                                     perf_mode=DR)
                nc.scalar.activation(h2b[:, m, s], pt[:], RELU,
                                     bias=b2sb[:, m:m + 1], scale=D2)
                nc.scalar.activation(h28[:, m, s], pt[:], RELU,
                                     bias=b2ss[:, m:m + 1], scale=D2 * S2)

        # ---- fc3 tap (bf16 from h2b) ----
        for nb in range(NBLK):
            s = slice(nb * NB, (nb + 1) * NB)
            pt = psum.tile([128, NB], F32)
            for k in range(8):
                nc.tensor.matmul(pt[:1], f3w[:, k:k + 1], h2b[:, k, s],
                                 start=(k == 0), stop=(k == 7))
            nc.scalar.activation(f3t[:1, s], pt[:1], RELU, bias=fb[2][:1])
        nc.sync.dma_start(cat[2:3, :], f3t[0:1, :])

        # ---- layer 3 main (fp8 DoubleRow): h2 [1024,BC] -> h3 [512,BC] ----
        h28v = h28.rearrange("p (a b) n -> p a b n", b=2)
        w3v = w3sb.rearrange("p m (a b c) -> p m a b c", b=2, c=128)
        for m in range(4):
            for nb in range(NBLK):
                s = slice(nb * NB, (nb + 1) * NB)
                pt = psum.tile([128, NB], F32)
                for kp in range(4):
                    nc.tensor.matmul(pt[:], w3v[:, m, kp], h28v[:, kp, :, s],
                                     start=(kp == 0), stop=(kp == 3),
                                     perf_mode=DR)
                nc.scalar.activation(h3b[:, m, s], pt[:], RELU,
                                     bias=b3sb[:, m:m + 1], scale=D3)

        # ---- fc4 tap (bf16 from h3b) ----
        for nb in range(NBLK):
            s = slice(nb * NB, (nb + 1) * NB)
            pt = psum.tile([128, NB], F32)
            for k in range(4):
                nc.tensor.matmul(pt[:1], f4w[:, k:k + 1], h3b[:, k, s],
                                 start=(k == 0), stop=(k == 3))
            nc.scalar.activation(f4t[:1, s], pt[:1], RELU, bias=fb[3][:1])

        # ---- readout ----
        nc.sync.dma_start(cat[3:4, :], f4t[0:1, :])
        for nb in range(NBLK):
            s = slice(nb * NB, (nb + 1) * NB)
            pt = psum.tile([128, NB], F32)
            nc.tensor.matmul(pt[:1], rwsb[:], cat[:, s], start=True, stop=True)
            nc.vector.tensor_scalar_add(outsb[:1, s], pt[:1], rbsb[:1])
        nc.sync.dma_start(out_d[:], outsb[:1, :])


_NC_CACHE = None


def _get_program():
    global _NC_CACHE
    if _NC_CACHE is None:
        _NC_CACHE = _build_program()
    return _NC_CACHE


def _prepare_in_maps(inputs):
    x = np.asarray(inputs["x"], np.float32)
    w1d = _densify(inputs["sl1_w"], inputs["sl1_out"], inputs["sl1_in"], L1, L2)
    w2d = _densify(inputs["sl2_w"], inputs["sl2_out"], inputs["sl2_in"], L2, L3)
    w3d = _densify(inputs["sl3_w"], inputs["sl3_out"], inputs["sl3_in"], L3, L4)
    b1 = _pack_b(inputs["sl1_b"], L2)
    b2 = _pack_b(inputs["sl2_b"], L3)
    w1pk = _pack_w(w1d, L1, L2)
    shared = {
        "w2p": _to_f8(_pack_w(w2d, L2, L3), SW2),
        "w3p": _to_f8(_pack_w(w3d, L3, L4), SW3),
        "f1w": _pack_fc(inputs["fc1_w"], L1),
        "f2w": _pack_fc(inputs["fc2_w"], L2),
        "f3w": _pack_fc(inputs["fc3_w"], L3),
        "f4w": _pack_fc(inputs["fc4_w"], L4),
        "b1": b1, "b1s": b1 * S1,
        "b2": b2, "b2s": b2 * S2,
        "b3": _pack_b(inputs["sl3_b"], L4),
        "fb1": np.asarray(inputs["fc1_b"], np.float32).reshape(1, 1),
        "fb2": np.asarray(inputs["fc2_b"], np.float32).reshape(1, 1),
        "fb3": np.asarray(inputs["fc3_b"], np.float32).reshape(1, 1),
        "fb4": np.asarray(inputs["fc4_b"], np.float32).reshape(1, 1),
        "rw": np.asarray(inputs["ro_w"], np.float32).reshape(4, 1).astype(NP_BF16),
        "rb": np.asarray(inputs["ro_b"], np.float32).reshape(1, 1),
    }
    if KF8 > 0:
        shared["w1p8"] = _to_f8(w1pk[:, :, :KF8 * 128], SW1)
    if KF8 < 32:
        shared["w1pb"] = np.ascontiguousarray(
            w1pk[:, :, KF8 * 128:] * (SX * SW1)).astype(NP_BF16)
    in_maps = []
    for c in range(NCORES):
        xt = np.ascontiguousarray(x[c * BC:(c + 1) * BC, :].T)
        xtr = np.ascontiguousarray(xt.reshape(32, 128, BC))
        per = {"xb": xtr.astype(NP_BF16), **shared}
        if KF8 > 0:
            per["x8"] = _to_f8(xtr[:KF8], SX)
        in_maps.append(per)
    return in_maps


def run(inputs, **kw):
    nc = _get_program()
    in_maps = _prepare_in_maps(inputs)
    res = bass_utils.run_bass_kernel_spmd(
        nc, in_maps, core_ids=list(range(NCORES)), **kw)
    out = np.concatenate([res.results[c]["out"].reshape(BC)
                          for c in range(NCORES)])
    return out.reshape(B, 1), res


def kernel(**inputs) -> np.ndarray:
    out, _ = run(inputs)
    return out


# revision 20
# speedup vs baseline: 2.1534x; 1.0061x over previous
"""Trainium2 Bass kernel for the HNN sparse-MLP network.

Strategy: densify the sparse edge lists into dense matrices on the host
and run the network as dense matmuls on the tensor engine, data-parallel
over the batch across 8 NeuronCores (1024 rows each).

Activations live feature-on-partition ([features, batch]) throughout:
    h_out[fo, b] = relu( sum_fi W[fi, fo] . h_in[fi, b] + bias )

Precision plan (rel tol 2e-2, measured 1.722e-2, bit-deterministic and
matching the numpy simulation of the same quantization to 0.1%):
  - layer-1/2/3 mains: fp8 e4m3 with DoubleRow perf mode (2 K-planes per
    pass = 2x bf16 throughput). KF8 selects how many of layer-1's 32
    k-tiles run fp8 (the rest bf16, sharing the psum group via bf16
    weights pre-scaled by the same pow2 factor) - a fallback dial if the
    error budget ever tightens: KF8=16 -> ~1.2e-2, KF8=0 -> ~3.4e-3.
  - h1, h2 are stored twice by dual activation passes per output tile:
    bf16 (feeding the fc taps) and scaled fp8 (feeding the next main).
  - fc taps f1..f4 + readout: bf16 (taps are 4096/2048/1024/512-long dot
    products whose error hits the output directly; fp8 there measurably
    fails the tolerance, e.g. an fp8 f1 tap alone adds ~2e-2).

h1/h2/h3 stay SBUF-resident (no DRAM spill); weights stream per m-tile
with double buffering.
"""

import sys

sys.path.insert(0, "/opt/trn_rl_repo")

import numpy as np
import ml_dtypes

import concourse.bass as bass
import concourse.tile as tile
import concourse.mybir as mybir
from concourse import bacc, bass_utils

F32 = mybir.dt.float32
BF16 = mybir.dt.bfloat16
F8 = mybir.dt.float8e4
RELU = mybir.ActivationFunctionType.Relu
DR = mybir.MatmulPerfMode.DoubleRow

NP_BF16 = ml_dtypes.bfloat16
NP_F8 = ml_dtypes.float8_e4m3

NCORES = 8
B, L1, L2, L3, L4 = 8192, 4096, 2048, 1024, 512
BC = B // NCORES          # batch rows per core
NB = 512                  # matmul moving free dim (PSUM bank = 512 fp32)
NBLK = BC // NB

# fp8 scales (powers of two; descale folds into the activation)
S1 = 16.0                 # h1 fp8 storage scale (h1 max ~1.0)
S2 = 64.0                 # h2 fp8 storage scale (h2 max ~0.16)
SW2 = 64.0                # w2 fp8 scale
SW3 = 64.0                # w3 fp8 scale
D2 = 1.0 / (S1 * SW2)     # layer-2 psum descale
D3 = 1.0 / (S2 * SW3)     # layer-3 psum descale

# layer-1 K-split: first KF8 of 32 k-tiles run fp8 e4m3 DoubleRow (2x),
# the rest bf16. The bf16 weights are pre-scaled by SX*SW1 (exact pow2)
# so both parts share one psum accumulation group and one descale.
KF8 = 32
SX = 8.0                  # x fp8 scale
SW1 = 64.0                # w1 fp8 scale
D1 = 1.0 / (SX * SW1)     # layer-1 psum descale


def _densify(w, out_idx, in_idx, in_dim, out_dim):
    wd = np.zeros((in_dim, out_dim), np.float32)
    np.add.at(wd, (np.asarray(in_idx), np.asarray(out_idx)),
              np.asarray(w, np.float32))
    return wd


def _pack_w(wd, in_dim, out_dim):
    """[in_dim, out_dim] -> [T, 128, in_dim] with
    wp[t, p, j*128+m] = wd[j*128+p, t*128+m]."""
    kt, t = in_dim // 128, out_dim // 128
    return np.ascontiguousarray(
        wd.reshape(kt, 128, t, 128).transpose(2, 1, 0, 3).reshape(t, 128, in_dim))


def _to_f8(a, s):
    a = np.asarray(a, np.float32) * s
    assert np.abs(a).max() < 224.0, "fp8 overflow risk"
    return a.astype(NP_F8)


def _pack_b(b, out_dim):
    t = out_dim // 128
    return np.ascontiguousarray(np.asarray(b, np.float32).reshape(t, 128).T)


def _pack_fc(w, in_dim):
    """fc row [1, in_dim] -> [128, kt] bf16 (k-tile per column)."""
    kt = in_dim // 128
    return np.ascontiguousarray(
        np.asarray(w, np.float32).reshape(kt, 128).T).astype(NP_BF16)


def _build_program():
    nc = bacc.Bacc("TRN2", target_bir_lowering=False, debug=False,
                   num_devices=NCORES)
    d = {}
    d["xb"] = nc.dram_tensor("xb", [32, 128, BC], BF16, kind="ExternalInput").ap()
    if KF8 > 0:
        d["x8"] = nc.dram_tensor("x8", [KF8, 128, BC], F8, kind="ExternalInput").ap()
        d["w1p8"] = nc.dram_tensor("w1p8", [16, 128, KF8 * 128], F8,
                                   kind="ExternalInput").ap()
    if KF8 < 32:
        d["w1pb"] = nc.dram_tensor("w1pb", [16, 128, (32 - KF8) * 128], BF16,
                                   kind="ExternalInput").ap()
    d["w2p"] = nc.dram_tensor("w2p", [8, 128, L2], F8, kind="ExternalInput").ap()
    d["w3p"] = nc.dram_tensor("w3p", [4, 128, L3], F8, kind="ExternalInput").ap()
    d["f1w"] = nc.dram_tensor("f1w", [128, 32], BF16, kind="ExternalInput").ap()
    d["f2w"] = nc.dram_tensor("f2w", [128, 16], BF16, kind="ExternalInput").ap()
    d["f3w"] = nc.dram_tensor("f3w", [128, 8], BF16, kind="ExternalInput").ap()
    d["f4w"] = nc.dram_tensor("f4w", [128, 4], BF16, kind="ExternalInput").ap()
    d["b1"] = nc.dram_tensor("b1", [128, 16], F32, kind="ExternalInput").ap()
    d["b1s"] = nc.dram_tensor("b1s", [128, 16], F32, kind="ExternalInput").ap()
    d["b2"] = nc.dram_tensor("b2", [128, 8], F32, kind="ExternalInput").ap()
    d["b2s"] = nc.dram_tensor("b2s", [128, 8], F32, kind="ExternalInput").ap()
    d["b3"] = nc.dram_tensor("b3", [128, 4], F32, kind="ExternalInput").ap()
    for i in range(1, 5):
        d[f"fb{i}"] = nc.dram_tensor(f"fb{i}", [1, 1], F32, kind="ExternalInput").ap()
    d["rw"] = nc.dram_tensor("rw", [4, 1], BF16, kind="ExternalInput").ap()
    d["rb"] = nc.dram_tensor("rb", [1, 1], F32, kind="ExternalInput").ap()
    out_d = nc.dram_tensor("out", [1, BC], F32, kind="ExternalOutput").ap()

    with tile.TileContext(nc) as tc:
        _emit(nc, tc, d, out_d)
    nc.compile()
    return nc


def _emit(nc, tc, d, out_d):
    from contextlib import ExitStack

    with ExitStack() as ctx:
        consts = ctx.enter_context(tc.tile_pool(name="consts", bufs=1))
        psum = ctx.enter_context(tc.tile_pool(name="psum", bufs=4, space="PSUM"))

        def cload(name, shape, dt):
            t = consts.tile(shape, dt, tag=name)
            nc.sync.dma_start(t[:], d[name][:])
            return t

        # persistent activations + preloaded small weights
        h1b = consts.tile([128, 16, BC], BF16, tag="h1b")
        h18 = consts.tile([128, 16, BC], F8, tag="h18")
        h2b = consts.tile([128, 8, BC], BF16, tag="h2b")
        h28 = consts.tile([128, 8, BC], F8, tag="h28")
        h3b = consts.tile([128, 4, BC], BF16, tag="h3b")
        f1t = consts.tile([1, BC], BF16, tag="f1t")
        f2t = consts.tile([1, BC], BF16, tag="f2t")
        f3t = consts.tile([1, BC], BF16, tag="f3t")
        f4t = consts.tile([1, BC], BF16, tag="f4t")
        cat = consts.tile([4, BC], BF16, tag="cat")
        outsb = consts.tile([1, BC], F32, tag="outsb")

        with tc.tile_pool(name="xp", bufs=1) as xpool, \
             tc.tile_pool(name="w1m", bufs=4) as w1pool:
            # DMA priority: m=0 weights first (gates the first matmul), then
            # x tiles split across both DGE engines, then small consts.
            # w2/w3 are loaded after the L1 loop - they aren't needed until
            # late and would delay the critical path here.
            wts = {}

            def w1_load(m):
                parts = []
                if KF8 > 0:
                    w8 = w1pool.tile([128, KF8 * 128], F8, tag="w1m8")
                    half = KF8 * 64
                    nc.sync.dma_start(w8[:, :half], d["w1p8"][m, :, :half])
                    nc.scalar.dma_start(w8[:, half:], d["w1p8"][m, :, half:])
                    parts.append(w8)
                else:
                    parts.append(None)
                if KF8 < 32:
                    wb = w1pool.tile([128, (32 - KF8) * 128], BF16, tag="w1mb")
                    nc.scalar.dma_start(wb[:], d["w1pb"][m])
                    parts.append(wb)
                else:
                    parts.append(None)
                wts[m] = parts

            w1_load(0)
            xb = xpool.tile([128, 32, BC], BF16, tag="xb")
            xview = d["xb"]
            if KF8 > 0:
                x8 = xpool.tile([128, KF8, BC], F8, tag="x8")
                for j in range(KF8):
                    eng = nc.sync if j % 2 == 0 else nc.scalar
                    eng.dma_start(x8[:, j, :], d["x8"][j])
                x8v = x8.rearrange("p (a b) n -> p a b n", b=2)

            def xb_load():
                for j in range(32):
                    eng = nc.sync if j % 2 == 0 else nc.scalar
                    eng.dma_start(xb[:, j, :], xview[j])

            if KF8 < 32:
                # bf16 part of L1 reads xb from the first m-tile on
                xb_load()
            w1_load(1)

            b1sb = cload("b1", [128, 16], F32)
            b1ss = cload("b1s", [128, 16], F32)
            b2sb = cload("b2", [128, 8], F32)
            b2ss = cload("b2s", [128, 8], F32)
            b3sb = cload("b3", [128, 4], F32)
            f1w = cload("f1w", [128, 32], BF16)
            f2w = cload("f2w", [128, 16], BF16)
            f3w = cload("f3w", [128, 8], BF16)
            f4w = cload("f4w", [128, 4], BF16)
            fb = [cload(f"fb{i}", [1, 1], F32) for i in range(1, 5)]
            rwsb = cload("rw", [4, 1], BF16)
            rbsb = cload("rb", [1, 1], F32)
            w1_load(2)

            # ---- layer 1 main: x [4096,BC] -> h1 [2048,BC] ----
            # k-tiles [0,KF8) fp8 DoubleRow, [KF8,32) bf16, one psum group
            for m in range(16):
                if m not in wts:
                    w1_load(m)
                w8t, wbt = wts.pop(m)
                if m + 3 <= 15 and m + 3 not in wts:
                    w1_load(m + 3)
                if w8t is not None:
                    w8v = w8t.rearrange("p (a b c) -> p a b c", b=2, c=128)
                for nb in range(NBLK):
                    s = slice(nb * NB, (nb + 1) * NB)
                    pt = psum.tile([128, NB], F32)
                    for kp in range(KF8 // 2):
                        nc.tensor.matmul(pt[:], w8v[:, kp], x8v[:, kp, :, s],
                                         start=(kp == 0),
                                         stop=(KF8 == 32 and kp == 15),
                                         perf_mode=DR)
                    for k in range(KF8, 32):
                        nc.tensor.matmul(
                            pt[:], wbt[:, (k - KF8) * 128:(k - KF8 + 1) * 128],
                            xb[:, k, s],
                            start=(k == KF8 == 0), stop=(k == 31))
                    nc.scalar.activation(h1b[:, m, s], pt[:], RELU,
                                         bias=b1sb[:, m:m + 1], scale=D1)
                    nc.scalar.activation(h18[:, m, s], pt[:], RELU,
                                         bias=b1ss[:, m:m + 1], scale=D1 * S1)

            if KF8 == 32:
                # xb only feeds the fc1 tap - load after the L1 weight
                # stream so it doesn't delay the critical path
                xb_load()

            # ---- fc1 tap (bf16): f1 = relu(fc1 . x) ----
            for nb in range(NBLK):
                s = slice(nb * NB, (nb + 1) * NB)
                pt = psum.tile([128, NB], F32)
                for k in range(32):
                    nc.tensor.matmul(pt[:1], f1w[:, k:k + 1], xb[:, k, s],
                                     start=(k == 0), stop=(k == 31))
                nc.scalar.activation(f1t[:1, s], pt[:1], RELU, bias=fb[0][:1])
            nc.sync.dma_start(cat[0:1, :], f1t[0:1, :])

        # layer-2/3 weights: small, needed only after L1 - loaded late (no
        # contention with the startup-critical x/w1 transfers) and allocated
        # after the x pool closes so they reuse its SBUF space
        l23 = ctx.enter_context(tc.tile_pool(name="l23w", bufs=1))
        w2sb = l23.tile([128, 8, L2], F8, tag="w2sb")
        for m in range(8):
            nc.sync.dma_start(w2sb[:, m, :], d["w2p"][m])
        w3sb = l23.tile([128, 4, L3], F8, tag="w3sb")
        for m in range(4):
            nc.sync.dma_start(w3sb[:, m, :], d["w3p"][m])

        # ---- fc2 tap (bf16 from h1b) ----
        for nb in range(NBLK):
            s = slice(nb * NB, (nb + 1) * NB)
            pt = psum.tile([128, NB], F32)
            for k in range(16):
                nc.tensor.matmul(pt[:1], f2w[:, k:k + 1], h1b[:, k, s],
                                 start=(k == 0), stop=(k == 15))
            nc.scalar.activation(f2t[:1, s], pt[:1], RELU, bias=fb[1][:1])
        nc.sync.dma_start(cat[1:2, :], f2t[0:1, :])

        # ---- layer 2 main (fp8 DoubleRow): h1 [2048,BC] -> h2 [1024,BC] ----
        h18v = h18.rearrange("p (a b) n -> p a b n", b=2)
        w2v = w2sb.rearrange("p m (a b c) -> p m a b c", b=2, c=128)
        for m in range(8):
            for nb in range(NBLK):
                s = slice(nb * NB, (nb + 1) * NB)
                pt = psum.tile([128, NB], F32)
                for kp in range(8):
                    nc.tensor.matmul(pt[:], w2v[:, m, kp], h18v[:, kp, :, s],
                                     start=(kp == 0), stop=(kp == 7),
                                     perf_mode=DR)
                nc.scalar.activation(h2b[:, m, s], pt[:], RELU,
                                     bias=b2sb[:, m:m + 1], scale=D2)
                nc.scalar.activation(h28[:, m, s], pt[:], RELU,
                                     bias=b2ss[:, m:m + 1], scale=D2 * S2)

        # ---- fc3 tap (bf16 from h2b) ----
        for nb in range(NBLK):
            s = slice(nb * NB, (nb + 1) * NB)
            pt = psum.tile([128, NB], F32)
            for k in range(8):
                nc.tensor.matmul(pt[:1], f3w[:, k:k + 1], h2b[:, k, s],
                                 start=(k == 0), stop=(k == 7))
            nc.scalar.activation(f3t[:1, s], pt[:1], RELU, bias=fb[2][:1])
        nc.sync.dma_start(cat[2:3, :], f3t[0:1, :])

        # ---- layer 3 main (fp8 DoubleRow): h2 [1024,BC] -> h3 [512,BC] ----
        h28v = h28.rearrange("p (a b) n -> p a b n", b=2)
        w3v = w3sb.rearrange("p m (a b c) -> p m a b c", b=2, c=128)
        for m in range(4):
            for nb in range(NBLK):
                s = slice(nb * NB, (nb + 1) * NB)
                pt = psum.tile([128, NB], F32)
                for kp in range(4):
                    nc.tensor.matmul(pt[:], w3v[:, m, kp], h28v[:, kp, :, s],
                                     start=(kp == 0), stop=(kp == 3),
                                     perf_mode=DR)
                nc.scalar.activation(h3b[:, m, s], pt[:], RELU,
                                     bias=b3sb[:, m:m + 1], scale=D3)

        # ---- fc4 tap (bf16 from h3b) ----
        for nb in range(NBLK):
            s = slice(nb * NB, (nb + 1) * NB)
            pt = psum.tile([128, NB], F32)
            for k in range(4):
                nc.tensor.matmul(pt[:1], f4w[:, k:k + 1], h3b[:, k, s],
                                 start=(k == 0), stop=(k == 3))
            nc.scalar.activation(f4t[:1, s], pt[:1], RELU, bias=fb[3][:1])

        # ---- readout ----
        nc.sync.dma_start(cat[3:4, :], f4t[0:1, :])
        for nb in range(NBLK):
            s = slice(nb * NB, (nb + 1) * NB)
            pt = psum.tile([128, NB], F32)
            nc.tensor.matmul(pt[:1], rwsb[:], cat[:, s], start=True, stop=True)
            nc.vector.tensor_scalar_add(outsb[:1, s], pt[:1], rbsb[:1])
        nc.sync.dma_start(out_d[:], outsb[:1, :])


_NC_CACHE = None


def _get_program():
    global _NC_CACHE
    if _NC_CACHE is None:
        _NC_CACHE = _build_program()
    return _NC_CACHE


def _prepare_in_maps(inputs):
    x = np.asarray(inputs["x"], np.float32)
    w1d = _densify(inputs["sl1_w"], inputs["sl1_out"], inputs["sl1_in"], L1, L2)
    w2d = _densify(inputs["sl2_w"], inputs["sl2_out"], inputs["sl2_in"], L2, L3)
    w3d = _densify(inputs["sl3_w"], inputs["sl3_out"], inputs["sl3_in"], L3, L4)
    b1 = _pack_b(inputs["sl1_b"], L2)
    b2 = _pack_b(inputs["sl2_b"], L3)
    w1pk = _pack_w(w1d, L1, L2)
    shared = {
        "w2p": _to_f8(_pack_w(w2d, L2, L3), SW2),
        "w3p": _to_f8(_pack_w(w3d, L3, L4), SW3),
        "f1w": _pack_fc(inputs["fc1_w"], L1),
        "f2w": _pack_fc(inputs["fc2_w"], L2),
        "f3w": _pack_fc(inputs["fc3_w"], L3),
        "f4w": _pack_fc(inputs["fc4_w"], L4),
        "b1": b1, "b1s": b1 * S1,
        "b2": b2, "b2s": b2 * S2,
        "b3": _pack_b(inputs["sl3_b"], L4),
        "fb1": np.asarray(inputs["fc1_b"], np.float32).reshape(1, 1),
        "fb2": np.asarray(inputs["fc2_b"], np.float32).reshape(1, 1),
        "fb3": np.asarray(inputs["fc3_b"], np.float32).reshape(1, 1),
        "fb4": np.asarray(inputs["fc4_b"], np.float32).reshape(1, 1),
        "rw": np.asarray(inputs["ro_w"], np.float32).reshape(4, 1).astype(NP_BF16),
        "rb": np.asarray(inputs["ro_b"], np.float32).reshape(1, 1),
    }
    if KF8 > 0:
        shared["w1p8"] = _to_f8(w1pk[:, :, :KF8 * 128], SW1)
    if KF8 < 32:
        shared["w1pb"] = np.ascontiguousarray(
            w1pk[:, :, KF8 * 128:] * (SX * SW1)).astype(NP_BF16)
    in_maps = []
    for c in range(NCORES):
        xt = np.ascontiguousarray(x[c * BC:(c + 1) * BC, :].T)
        xtr = np.ascontiguousarray(xt.reshape(32, 128, BC))
        per = {"xb": xtr.astype(NP_BF16), **shared}
        if KF8 > 0:
            per["x8"] = _to_f8(xtr[:KF8], SX)
        in_maps.append(per)
    return in_maps


def run(inputs, **kw):
    nc = _get_program()
    in_maps = _prepare_in_maps(inputs)
    res = bass_utils.run_bass_kernel_spmd(
        nc, in_maps, core_ids=list(range(NCORES)), **kw)
    out = np.concatenate([res.results[c]["out"].reshape(BC)
                          for c in range(NCORES)])
    return out.reshape(B, 1), res


def kernel(**inputs) -> np.ndarray:
    out, _ = run(inputs)
    return out
